# revision 1
# baseline (speedup 1.0000x reference)
"""Trainium2 Bass kernel for nn_MultiHeadAttention (B=4, T=1024, D=1024, H=16, dk=64).

Sharding: 8 cores = 4 batches x 2 head-groups (8 heads / 512 features each).
Each core computes a partial output (its head-group's contribution through Wo);
host sums the two partials per batch (the "all-reduce after linear_out" done
host-side during unshard) and adds bo.

Per-core dataflow (all on one NeuronCore, Tile-scheduled):
  A) q/k/v projections as X^T-major fp32r matmuls -> (Tq partitions, F free),
     drain + per-head LayerNorm (bn_stats, manual even/odd aggregation,
     normalize on GPSIMD -> bf16 qhat), PE-transpose 128x128 blocks into
     qlnT/klnT (F partitions, T free) with gamma (and 1/sqrt(dk) for q)
     applied per-partition on the PSUM drain.
  B) per head: scoresT = klnT-slice.T @ qlnT-slice (K=64, head pairs packed
     into PE row groups 0-63/64-127), exp on ACT (PSUM [128,1024] -> SBUF
     bf16), mask multiply (bf16 DVE), x_aug = [v|1].T @ attnT accumulated
     over Tk chunks -> 64 rows of x + 64 rows of broadcast denominator.
     Reciprocal of denom via ACT exp(-ln d) (c=0) / exact DVE recip (c=1),
     DMA partition-shift to the x rows, multiply -> x_all (F part, Tq).
  C) out = x_all-slices.T @ WoT (fp32r) -> (T, 1024) partial, DMA out.

Why the odd bits: walrus here allows only ONE sync-wait per instruction
(_split_excess_waits patches the BIR); custom-DVE reciprocal_approx and
SBUF->SBUF DMA-transpose are broken in this toolchain (see memory notes).
"""

import os
import numpy as np
import ml_dtypes

T = 1024
D = 1024
F = 512      # features per core (8 heads x 64)
NH = 8       # heads per core
DK = 64
P = 128
EPS = 1e-5
BF16 = ml_dtypes.bfloat16

_CACHE = {}

# feature knobs (for bisecting compiler issues)
USE_DMA_TRANSPOSE = False  # SBUF->SBUF xbar transpose corrupts under concurrent DMA traffic
RECIP_MODE = "split"  # "lnexp" (ACT), "exact" (DVE), "split" (both)
DEBUG = False  # add intermediate dumps as extra outputs
USE_GPSIMD_MASK = True
MASK_GPSIMD_MOD = 0   # tk % MOD == MOD-1 goes to gpsimd; 0 disables
SC_BUFS = 2
PS512_BUFS = 4
ATTN_BUFS = 6
PHASES = 3  # 1=A only, 2=A+B, 3=full
SWDGE_LOADS = True  # route v/mask/wo loads through gpsimd SWDGE queues
FAKE_XT = False  # replace xt DMA loads with memsets (sim experiment)
NORM_ENGINE = "vector"  # "gpsimd" or "vector"
DRAIN_BUFS = 4
STAT_BUFS = 4
QHAT_BUFS = 3
INTERLEAVE_KQ = False
MASK_PER_C = True
PDRAIN_ENGINE = "any"
C_OUTER = False
PE_SHIFT = False
V_ORDER = "last"
V_INTERLEAVE = False
PST_POOL = "sc"
XIN_BUFS = 4
GB_ALT = False
PE_SHIFT_LAST = False
A_STOP = 4  # 1=proj+drain 2=+stats 3=+normalize 4=full A


def _split_excess_waits(bj):
    """Walrus allows at most 1 sync-wait per instruction (2 for
    EventSemaphore). Tile's sem assigner can emit more; spill the excess
    onto NoOp carriers inserted just before, on the same engine."""
    import json
    d = json.loads(bj)
    ctr = 0
    for fn in d["functions"]:
        for bb in fn["blocks"]:
            new = []
            for inst in bb["instructions"]:
                si = inst.get("sync_info") or {}
                ow = si.get("on_wait") or []
                op = inst.get("opcode", "")
                cap = 2 if op == "EventSemaphore" else 1
                if len(ow) > cap:
                    for w in ow[:-cap]:
                        ctr += 1
                        new.append({
                            "debug": inst.get("debug", 0),
                            "engine": inst["engine"],
                            "ins": [], "outs": [],
                            "name": f"W-{ctr}",
                            "opcode": "NoOp",
                            "sync_info": {"on_update": [], "on_wait": [w]},
                            "text_hint": "waitsplit",
                        })
                    si["on_wait"] = ow[-cap:]
                new.append(inst)
            bb["instructions"] = new
    return json.dumps(d).encode(), ctr


def _build(use_bq, use_bk, use_bv, ln_beta_zero=True):
    import concourse.bass as bass
    import concourse.tile as tile
    from concourse import mybir

    f32 = mybir.dt.float32
    f32r = mybir.dt.float32r
    bf16 = mybir.dt.bfloat16

    nc = bass.Bass()


    # ---- DRAM I/O ----
    xq_t = nc.dram_tensor("xq_t", (D, T), f32r, kind="ExternalInput").ap()
    xk_t = nc.dram_tensor("xk_t", (D, T), f32r, kind="ExternalInput").ap()
    xv_t = nc.dram_tensor("xv_t", (D, T), f32r, kind="ExternalInput").ap()
    wq_t = nc.dram_tensor("wq_t", (D, F), f32r, kind="ExternalInput").ap()
    wk_t = nc.dram_tensor("wk_t", (D, F), f32r, kind="ExternalInput").ap()
    wv_t = nc.dram_tensor("wv_t", (D, F), f32r, kind="ExternalInput").ap()
    wo_t = nc.dram_tensor("wo_t", (F, D), f32r, kind="ExternalInput").ap()
    mask_t = nc.dram_tensor("mask_t", (T, T), bf16, kind="ExternalInput").ap()
    # per-partition LN constants (128,) = per (head-pair-local feature)
    gq_d = nc.dram_tensor("gq", (P, 1), f32, kind="ExternalInput").ap()
    bq_d = nc.dram_tensor("bq_ln", (P, 1), f32, kind="ExternalInput").ap()
    gk_d = nc.dram_tensor("gk", (P, 1), f32, kind="ExternalInput").ap()
    bk_d = nc.dram_tensor("bk_ln", (P, 1), f32, kind="ExternalInput").ap()
    biases = {}
    for name, used in (("bq", use_bq), ("bk", use_bk), ("bv", use_bv)):
        if used:
            biases[name] = nc.dram_tensor(name, (F,), f32, kind="ExternalInput").ap()
    if PE_SHIFT or PE_SHIFT_LAST:
        identr_d = nc.dram_tensor("identr_d", (P, P), f32r, kind="ExternalInput").ap()
    out_p = nc.dram_tensor("out_p", (T, D), f32, kind="ExternalOutput").ap()
    dbg = {}
    if DEBUG:
        dbg["qlnT"] = nc.dram_tensor("dbg_qlnT", (P, 4, T), bf16, kind="ExternalOutput").ap()
        dbg["klnT"] = nc.dram_tensor("dbg_klnT", (P, 4, T), bf16, kind="ExternalOutput").ap()
        dbg["vaug"] = nc.dram_tensor("dbg_vaug", (P, 8, NH, P), bf16, kind="ExternalOutput").ap()
        dbg["xall"] = nc.dram_tensor("dbg_xall", (P, 4, T), f32r, kind="ExternalOutput").ap()
        dbg["qsb0"] = nc.dram_tensor("dbg_qsb0", (P, NH, DK), f32, kind="ExternalOutput").ap()
        dbg["at00"] = nc.dram_tensor("dbg_at00", (P, T), bf16, kind="ExternalOutput").ap()

    # DRAM views
    xviews = {
        "q": xq_t.rearrange("(dc p) t -> p dc t", p=P),
        "k": xk_t.rearrange("(dc p) t -> p dc t", p=P),
        "v": xv_t.rearrange("(dc p) t -> p dc t", p=P),
    }
    wviews = {
        "q": wq_t.rearrange("(dc p) f -> p dc f", p=P),
        "k": wk_t.rearrange("(dc p) f -> p dc f", p=P),
        "v": wv_t.rearrange("(dc p) f -> p dc f", p=P),
    }
    wo_view = wo_t.rearrange("(fc p) d -> p fc d", p=P)
    mask_view = mask_t.rearrange("(kc p) t -> p kc t", p=P)
    out_view = out_p.rearrange("(tc p) d -> p tc d", p=P)

    with tile.TileContext(nc) as tc:
        with (
            tc.tile_pool(name="const", bufs=1) as const,
            tc.tile_pool(name="xin", bufs=XIN_BUFS) as xin,
            tc.tile_pool(name="drain", bufs=DRAIN_BUFS) as drain,
            tc.tile_pool(name="stat", bufs=STAT_BUFS) as stat,
            tc.tile_pool(name="qhatp", bufs=QHAT_BUFS) as qhatp,
            tc.tile_pool(name="attnp", bufs=ATTN_BUFS) as attnp,
            tc.tile_pool(name="recipp", bufs=3) as recipp,
            tc.tile_pool(name="outp", bufs=3) as outp,
            tc.tile_pool(name="ps512", bufs=PS512_BUFS, space="PSUM") as ps512,
            tc.tile_pool(name="ps1024", bufs=SC_BUFS, space="PSUM") as ps1024,
        ):
            # ---- resident tiles ----
            w_sb = {
                pn: const.tile([P, 8, F], f32r, name=f"w_{pn}", tag=f"w_{pn}") for pn in ("q", "k", "v")
            }
            wo_sb = const.tile([P, 4, D], f32r, name="wo", tag="wo")
            qlnT = const.tile([P, 4, T], bf16, name="qlnT", tag="qlnT")
            klnT = const.tile([P, 4, T], bf16, name="klnT", tag="klnT")
            vaug = const.tile([P, 8, NH, P], bf16, name="vaug", tag="vaug")  # [p, tk, h, 128]
            mask_sb = const.tile([P, 8, T], bf16, name="mask", tag="mask")
            x_all = const.tile([P, 4, T], f32r, name="xall", tag="xall")
            eps_t = const.tile([P, 1], f32, name="eps", tag="eps")
            gb_t = {}
            for nm, dr in (("gq", gq_d), ("bq", bq_d), ("gk", gk_d), ("bk", bk_d)):
                gb_t[nm] = const.tile([P, 1], f32, name=f"ln_{nm}", tag=f"ln_{nm}")
                nc.sync.dma_start(gb_t[nm], dr)
            nc.vector.memset(eps_t, EPS)
            from concourse.masks import make_identity
            ident = const.tile([P, P], bf16, name="ident", tag="ident")
            make_identity(nc, ident)
            if PE_SHIFT or PE_SHIFT_LAST:
                identr = const.tile([P, P], f32r, name="identr", tag="identr")
                nc.sync.dma_start(identr, identr_d)

            bias_bc = {}
            for name in biases:
                bias_bc[name] = const.tile([P, F], f32, name=f"bc_{name}", tag=f"bc_{name}")
                src = bass.AP(
                    tensor=biases[name].tensor,
                    offset=biases[name].offset,
                    ap=[[0, P], [1, F]],
                )
                nc.gpsimd.dma_start(out=bias_bc[name], in_=src)

            _dma2 = nc.gpsimd if SWDGE_LOADS else nc.sync
            for pn in ("q", "k", "v"):
                for d in range(8):
                    nc.sync.dma_start(w_sb[pn][:, d, :], wviews[pn][:, d, :])
            for j in range(4):
                _dma2.dma_start(wo_sb[:, j, :], wo_view[:, j, :])
            for tk in range(8):
                _dma2.dma_start(mask_sb[:, tk, :], mask_view[:, tk, :])
            # ones columns of v_aug: even h -> cols 64:128, odd h -> cols 0:64
            nc.gpsimd.memset(vaug[:, :, 0::2, DK:P], 1.0)
            nc.gpsimd.memset(vaug[:, :, 1::2, 0:DK], 1.0)

            ln_params = {"q": ("gq", "bq"), "k": ("gk", "bk")}

            # ---- Phase A: projections + LN + transpose ----
            def proj_ln(pn, dstT, t_list=None):
                bias_name = "b" + pn
                for t in (t_list if t_list is not None else range(8)):
                    xt = xin.tile([P, 8, P], f32r, name="xt", tag="xt")
                    if FAKE_XT:
                        nc.vector.memset(xt, 0.01)
                    else:
                        nc.sync.dma_start(xt, xviews[pn][:, :, t * P:(t + 1) * P])
                    ps = ps512.tile([P, F], f32, name="ps512", tag="ps512")
                    for d in range(8):
                        nc.tensor.matmul(
                            ps, lhsT=xt[:, d, :], rhs=w_sb[pn][:, d, :],
                            start=(d == 0), stop=(d == 7),
                        )
                    if A_STOP < 1:
                        continue
                    sb = drain.tile([P, NH, DK], f32, name="qsb", tag="qsb")
                    if bias_name in bias_bc:
                        nc.vector.tensor_add(
                            sb.rearrange("p h d -> p (h d)"), ps, bias_bc[bias_name])
                    else:
                        deng = nc.vector if PDRAIN_ENGINE == "vector" else nc.any
                        deng.tensor_copy(
                            out=sb.rearrange("p h d -> p (h d)"), in_=ps)
                    if A_STOP < 2:
                        continue
                    st = stat.tile([P, NH, 6], f32, name="st", tag="st")
                    for h in range(NH):
                        nc.vector.bn_stats(out=st[:, h, :], in_=sb[:, h, :])
                    # combine even/odd halves: mu=(me+mo)/2;
                    # var=(32ve+32vo)/64 + ((me-mo)/2)^2
                    me, mo = st[:, :, 1], st[:, :, 4]
                    ve, vo = st[:, :, 2], st[:, :, 5]
                    mu = stat.tile([P, NH], f32, name="mu", tag="mu")
                    nc.vector.tensor_add(mu, me, mo)
                    nc.vector.tensor_scalar_mul(mu, mu, 0.5)
                    dm = stat.tile([P, NH], f32, name="dm", tag="dm")
                    nc.vector.tensor_sub(dm, me, mo)
                    nc.vector.tensor_scalar_mul(dm, dm, 0.5)
                    nc.vector.tensor_mul(dm, dm, dm)  # ((me-mo)/2)^2
                    sv = stat.tile([P, NH], f32, name="sv", tag="sv")
                    nc.vector.tensor_add(sv, ve, vo)
                    var = stat.tile([P, NH], f32, name="var", tag="var")
                    # var = sv/64 + dm
                    nc.vector.scalar_tensor_tensor(
                        out=var, in0=sv, scalar=1.0 / DK, in1=dm,
                        op0=mybir.AluOpType.mult,
                        op1=mybir.AluOpType.add)
                    sd = stat.tile([P, NH], f32, name="sd", tag="sd")
                    nc.scalar.activation(
                        out=sd, in_=var,
                        func=mybir.ActivationFunctionType.Sqrt,
                        bias=eps_t,
                    )
                    rs = stat.tile([P, NH], f32, name="rs", tag="rs")
                    nc.vector.reciprocal(out=rs, in_=sd)
                    if DEBUG and pn == "q" and t == 0:
                        nc.sync.dma_start(out=dbg["qsb0"], in_=sb)
                    if A_STOP < 3:
                        continue
                    qh = qhatp.tile([P, F], bf16, name="qh", tag="qh")
                    norm_eng = nc.gpsimd if NORM_ENGINE == "gpsimd" else nc.vector
                    for h in range(NH):
                        norm_eng.tensor_scalar(
                            out=qh[:, h * DK:(h + 1) * DK],
                            in0=sb[:, h, :],
                            scalar1=mu[:, h:h + 1],
                            scalar2=rs[:, h:h + 1],
                            op0=mybir.AluOpType.subtract,
                            op1=mybir.AluOpType.mult,
                        )
                    if A_STOP < 4:
                        continue
                    g_nm, b_nm = ln_params[pn]
                    for j in range(4):
                        if PST_POOL == "ps512":
                            pst = ps512.tile([P, 2 * F], bf16, name="ps_bf", tag="ps512")
                        else:
                            pst = ps1024.tile([P, 2 * T], bf16, name="sc_bf", tag="sc")
                        nc.tensor.transpose(
                            pst[:, 0:P], qh[:, j * P:(j + 1) * P], ident)
                        if ln_beta_zero and not (GB_ALT and j % 2 == 1):
                            nc.scalar.activation(
                                out=dstT[:, j, t * P:(t + 1) * P], in_=pst[:, 0:P],
                                func=mybir.ActivationFunctionType.Copy,
                                scale=gb_t[g_nm],
                            )
                        else:
                            nc.vector.tensor_scalar(
                                out=dstT[:, j, t * P:(t + 1) * P], in0=pst[:, 0:P],
                                scalar1=gb_t[g_nm], scalar2=gb_t[b_nm],
                                op0=mybir.AluOpType.mult, op1=mybir.AluOpType.add,
                            )

            def v_proj(ts_list):
                for t in ts_list:
                    xt = xin.tile([P, 8, P], f32r, name="xt", tag="xt")
                    _dma2.dma_start(xt, xviews["v"][:, :, t * P:(t + 1) * P])
                    ps = ps512.tile([P, F], f32, name="ps512", tag="ps512")
                    for d in range(8):
                        nc.tensor.matmul(
                            ps, lhsT=xt[:, d, :], rhs=w_sb["v"][:, d, :],
                            start=(d == 0), stop=(d == 7),
                        )
                    ps_h = ps.rearrange("p (hp two d) -> p hp two d", two=2, d=DK)
                    if "bv" in bias_bc:
                        vb = drain.tile([P, NH, DK], f32, name="vsb", tag="vsb")
                        nc.vector.tensor_add(
                            vb.rearrange("p h d -> p (h d)"), ps, bias_bc["bv"])
                        vb_h = vb.rearrange("p (hp two) d -> p hp two d", two=2)
                        nc.any.tensor_copy(out=vaug[:, t, 0::2, 0:DK], in_=vb_h[:, :, 0, :])
                        nc.any.tensor_copy(out=vaug[:, t, 1::2, DK:P], in_=vb_h[:, :, 1, :])
                    else:
                        nc.any.tensor_copy(out=vaug[:, t, 0::2, 0:DK], in_=ps_h[:, :, 0, :])
                        nc.any.tensor_copy(out=vaug[:, t, 1::2, DK:P], in_=ps_h[:, :, 1, :])

            if C_OUTER:
                proj_ln("q", qlnT, [0, 1, 2, 3])
                proj_ln("k", klnT)
                v_proj(range(8))
            elif V_ORDER == "binterleave":
                proj_ln("k", klnT)
                proj_ln("q", qlnT)
            elif V_ORDER == "first":
                v_proj(range(8))
                proj_ln("k", klnT)
                proj_ln("q", qlnT)
            elif V_ORDER == "mid":
                proj_ln("k", klnT)
                v_proj(range(8))
                proj_ln("q", qlnT)
            else:
                proj_ln("k", klnT)
                proj_ln("q", qlnT)
                v_proj(range(8))
            # ---- Phase B: attention ----
            def b_pair(j, c):
                """Attention for head pair j over Tq half c (c-outer layout)."""
                xps = {}
                for hh in range(2):
                    xps[2 * j + hh] = ps512.tile([P, F], f32, name="ps512", tag="ps512")
                for tk in range(8):
                    for hh in range(2):
                        h = 2 * j + hh
                        rows = slice(hh * DK, (hh + 1) * DK)
                        sp = ps1024.tile([P, T], f32, name="sc", tag="sc")[:, 0:F]
                        nc.tensor.matmul(
                            sp, lhsT=klnT[rows, j, tk * P:(tk + 1) * P],
                            rhs=qlnT[rows, j, c * F:(c + 1) * F],
                            start=True, stop=True)
                        at = attnp.tile([P, F], bf16, name="attn", tag="attn")
                        nc.scalar.activation(
                            out=at, in_=sp, func=mybir.ActivationFunctionType.Exp)
                        nc.vector.tensor_mul(at, at, mask_sb[:, tk, c * F:(c + 1) * F])
                        if DEBUG and j == 0 and hh == 0 and tk == 0:
                            nc.sync.dma_start(out=dbg["at00"][:, c * F:(c + 1) * F], in_=at)
                        nc.tensor.matmul(
                            xps[h], lhsT=vaug[:, tk, h, :], rhs=at,
                            start=(tk == 0), stop=(tk == 7))
                for hh in range(2):
                    h = 2 * j + hh
                    xrows = slice(0, DK) if hh == 0 else slice(DK, P)
                    drows = slice(DK, P) if hh == 0 else slice(0, DK)
                    rc = recipp.tile([P, F], f32r, name="rc", tag="rc")
                    if RECIP_MODE == "lnexp" or (RECIP_MODE == "split" and c == 0):
                        lg = recipp.tile([P, F], f32r, name="lg", tag="lg")
                        nc.scalar.activation(
                            out=lg[drows], in_=xps[h][drows],
                            func=mybir.ActivationFunctionType.Ln)
                        nc.scalar.activation(
                            out=rc[drows], in_=lg[drows],
                            func=mybir.ActivationFunctionType.Exp, scale=-1.0)
                    else:
                        with nc.allow_low_precision(reason="f32r==f32 bits; recip of softmax denom"):
                            nc.vector.reciprocal(out=rc[drows], in_=xps[h][drows])
                    rsh = recipp.tile([P, F], f32r, name="rsh", tag="rsh")
                    nc.sync.dma_start(out=rsh[xrows], in_=rc[drows])
                    nc.vector.tensor_mul(
                        x_all[xrows, j, c * F:(c + 1) * F],
                        xps[h][xrows], rsh[xrows])

            def b_pair_full(j, with_v=False, pe_shift=False):
                xps = {}
                for hh in range(2):
                    h = 2 * j + hh
                    xps[h] = [ps512.tile([P, F], f32, name="ps512", tag="ps512")
                              for _ in range(2)]
                for tk in range(8):
                    if with_v:
                        v_proj([tk])
                    for hh in range(2):
                        h = 2 * j + hh
                        rows = slice(hh * DK, (hh + 1) * DK)
                        sp = ps1024.tile([P, T], f32, name="sc", tag="sc")
                        lt = klnT[rows, j, tk * P:(tk + 1) * P]
                        nc.tensor.matmul(sp[:, 0:F], lhsT=lt, rhs=qlnT[rows, j, 0:F],
                                         start=True, stop=True)
                        nc.tensor.matmul(sp[:, F:T], lhsT=lt, rhs=qlnT[rows, j, F:T],
                                         start=True, stop=True)
                        at = attnp.tile([P, T], bf16, name="attn_f", tag="attn")
                        nc.scalar.activation(
                            out=at, in_=sp, func=mybir.ActivationFunctionType.Exp)
                        for c in range(2):
                            nc.vector.tensor_mul(
                                at[:, c * F:(c + 1) * F], at[:, c * F:(c + 1) * F],
                                mask_sb[:, tk, c * F:(c + 1) * F])
                            nc.tensor.matmul(
                                xps[h][c], lhsT=vaug[:, tk, h, :],
                                rhs=at[:, c * F:(c + 1) * F],
                                start=(tk == 0), stop=(tk == 7))
                        if DEBUG and j == 0 and hh == 0 and tk == 0:
                            nc.sync.dma_start(out=dbg["at00"], in_=at)
                for hh in range(2):
                    h = 2 * j + hh
                    xrows = slice(0, DK) if hh == 0 else slice(DK, P)
                    drows = slice(DK, P) if hh == 0 else slice(0, DK)
                    for c in range(2):
                        rc = recipp.tile([P, F], f32r, name="rc", tag="rc")
                        if RECIP_MODE == "lnexp" or (RECIP_MODE == "split" and c == 0):
                            lg = recipp.tile([P, F], f32r, name="lg", tag="lg")
                            nc.scalar.activation(
                                out=lg[drows], in_=xps[h][c][drows],
                                func=mybir.ActivationFunctionType.Ln)
                            nc.scalar.activation(
                                out=rc[drows], in_=lg[drows],
                                func=mybir.ActivationFunctionType.Exp, scale=-1.0)
                        else:
                            with nc.allow_low_precision(reason="f32r==f32 bits; recip of softmax denom"):
                                nc.vector.reciprocal(out=rc[drows], in_=xps[h][c][drows])
                        if PE_SHIFT or pe_shift:
                            rps = ps1024.tile([P, T], f32, name="rps", tag="sc")[:, 0:F]
                            nc.tensor.matmul(
                                rps[xrows], lhsT=identr[drows, drows], rhs=rc[drows],
                                start=True, stop=True,
                                tile_position=(drows.start, xrows.start))
                            rsh = recipp.tile([P, F], f32r, name="rsh", tag="rsh")
                            nc.any.tensor_copy(out=rsh[xrows], in_=rps[xrows])
                        else:
                            rsh = recipp.tile([P, F], f32r, name="rsh", tag="rsh")
                            nc.sync.dma_start(out=rsh[xrows], in_=rc[drows])
                        nc.vector.tensor_mul(
                            x_all[xrows, j, c * F:(c + 1) * F],
                            xps[h][c][xrows], rsh[xrows])

            def c_group(t):
                for n in range(2):
                    ps = ps1024.tile([P, T], f32, name="sc_c", tag="sc")[:, 0:F]
                    for jj in range(4):
                        nc.tensor.matmul(
                            ps, lhsT=x_all[:, jj, t * P:(t + 1) * P],
                            rhs=wo_sb[:, jj, n * F:(n + 1) * F],
                            start=(jj == 0), stop=(jj == 3),
                        )
                    ob = outp.tile([P, F], f32, name="ob", tag="ob")
                    nc.any.tensor_copy(out=ob, in_=ps)
                    nc.sync.dma_start(out=out_view[:, t, n * F:(n + 1) * F], in_=ob)

            if PHASES >= 2:
                if C_OUTER:
                    for j in range(4):
                        proj_ln("q", qlnT, [4 + j])
                        b_pair(j, 0)
                    for j in range(4):
                        b_pair(j, 1)
                        if PHASES >= 3:
                            c_group(j)
                else:
                    for j in range(4):
                        b_pair_full(j, with_v=(V_INTERLEAVE and j == 0),
                                    pe_shift=(PE_SHIFT_LAST and j == 3))
                    if PHASES >= 3:
                        for t in range(4):
                            c_group(t)
            elif C_OUTER:
                proj_ln("q", qlnT, [4, 5, 6, 7])
            # ---- Phase C: output projection (second Tq half) ----
            if PHASES >= 3:
                for t in range(4, 8):
                    c_group(t)

    return nc


def _get_nc(flags):
    if len(flags) == 3:
        flags = (*flags, True)
    key = (flags, USE_DMA_TRANSPOSE, RECIP_MODE, USE_GPSIMD_MASK, DEBUG,
           MASK_GPSIMD_MOD, SC_BUFS, PS512_BUFS, ATTN_BUFS, PHASES, SWDGE_LOADS, A_STOP, FAKE_XT, NORM_ENGINE, DRAIN_BUFS, STAT_BUFS, QHAT_BUFS, INTERLEAVE_KQ, MASK_PER_C, PDRAIN_ENGINE, C_OUTER, PE_SHIFT, V_ORDER, V_INTERLEAVE, PST_POOL, XIN_BUFS, GB_ALT, PE_SHIFT_LAST)
    if key not in _CACHE:
        nc = _build(*flags)
        patched, _n = _split_excess_waits(nc.to_json_bytes())
        nc.to_json_bytes = lambda: patched
        _CACHE[key] = nc
    return _CACHE[key]


def kernel(query, key, value, mask, Wq, bq, Wk, bk, Wv, bv, Wo, bo,
           q_gamma, q_beta, k_gamma, k_beta, _trace=False):
    from concourse.bass_utils import run_bass_kernel_spmd

    query = np.ascontiguousarray(np.asarray(query, np.float32))
    key = np.ascontiguousarray(np.asarray(key, np.float32))
    value = np.ascontiguousarray(np.asarray(value, np.float32))
    mask = np.asarray(mask)
    Wq, Wk, Wv, Wo = (np.asarray(w, np.float32) for w in (Wq, Wk, Wv, Wo))
    bq, bk, bv, bo = (np.asarray(b, np.float32) for b in (bq, bk, bv, bo))
    q_gamma, q_beta, k_gamma, k_beta = (
        np.asarray(g, np.float32) for g in (q_gamma, q_beta, k_gamma, k_beta))

    B = query.shape[0]
    use_bq, use_bk, use_bv = (bool(np.any(b)) for b in (bq, bk, bv))
    ln_beta_zero = not (np.any(q_beta) or np.any(k_beta))
    nc = _get_nc((use_bq, use_bk, use_bv, ln_beta_zero))

    # host-side shard prep
    xqT = [np.ascontiguousarray(query[b].T) for b in range(B)]
    xkT = [np.ascontiguousarray(key[b].T) for b in range(B)]
    xvT = [np.ascontiguousarray(value[b].T) for b in range(B)]
    maskT = [np.ascontiguousarray((~mask[b]).T.astype(BF16)) for b in range(B)]
    gq8 = np.ascontiguousarray((np.tile(q_gamma, 2) / 8.0).reshape(P, 1))
    bq8 = np.ascontiguousarray((np.tile(q_beta, 2) / 8.0).reshape(P, 1))
    gk2 = np.ascontiguousarray(np.tile(k_gamma, 2).reshape(P, 1))
    bk2 = np.ascontiguousarray(np.tile(k_beta, 2).reshape(P, 1))

    in_maps = []
    for core in range(8):
        b, g = core // 2, core % 2
        sl = slice(g * F, (g + 1) * F)
        im = {
            "xq_t": xqT[b], "xk_t": xkT[b], "xv_t": xvT[b],
            **({"identr_d": np.ascontiguousarray(np.eye(P, dtype=np.float32))}
               if (PE_SHIFT or PE_SHIFT_LAST) else {}),
            "wq_t": np.ascontiguousarray(Wq[sl].T),
            "wk_t": np.ascontiguousarray(Wk[sl].T),
            "wv_t": np.ascontiguousarray(Wv[sl].T),
            "wo_t": np.ascontiguousarray(Wo[:, sl].T),
            "mask_t": maskT[b],
            "gq": gq8, "bq_ln": bq8, "gk": gk2, "bk_ln": bk2,
        }
        if use_bq:
            im["bq"] = np.ascontiguousarray(bq[sl])
        if use_bk:
            im["bk"] = np.ascontiguousarray(bk[sl])
        if use_bv:
            im["bv"] = np.ascontiguousarray(bv[sl])
        in_maps.append(im)

    res = run_bass_kernel_spmd(nc, in_maps, core_ids=list(range(8)), trace=_trace)
    out = np.zeros((B, T, D), np.float32)
    for b in range(B):
        out[b] = res.results[2 * b]["out_p"] + res.results[2 * b + 1]["out_p"] + bo
    if _trace:
        kernel._last_results = res
    return out



# revision 26
# speedup vs baseline: 1.3923x; 1.3923x over previous
"""Trainium2 Bass kernel for nn_MultiHeadAttention (B=4, T=1024, D=1024, H=16, dk=64).

Sharding: 8 cores = 4 batches x 2 head-groups (8 heads / 512 features each).
Each core computes a partial output (its head-group's contribution through Wo);
host sums the two partials per batch and adds bo.

Per-core dataflow (one NeuronCore, Tile-scheduled), cost-model-guided; all
matmuls bf16 (fp8 DoubleRow was tried and rejected: e4m3's 2.65% RMS lands
~6% on the output, over the 2e-2 gate):
  A) q/k projections (bf16, 8 d-chunk accumulation) -> LN per head
     (bn_stats + bn_aggr, Rsqrt(var+eps) on ACT) -> normalize (Pool) ->
     qh bf16 -> 4 PE transposes per tile into a [P,512] PSUM quad ->
     one ACT scale-drain (gamma, and 1/sqrt(dk) on q) into qlnT/klnT
     [pair-features x 4 pairs x T].
  B) head-major attention: scoresT per (h, tk-block) = klnT-slice.T @ qlnT
     (2x 512-chunk matmuls, K=64) -> PSUM [128,1024]; exp on ACT (the
     bottleneck engine of this phase - nothing else is scheduled on ACT
     here); mask multiply on DVE; attnV transposed: out partitions = query
     positions, rhs = v_sb[tk,h,:] = [v|1] (65 cols, ones column yields
     softmax denominators in col 64) accumulated over tk chunks into
     bank-sized PSUM [128,4,128] tiles; DVE reciprocal + per-qb
     tensor_scalar -> xT bf16; after each odd head 8 PE transposes ->
     x_all [f, T]. v-projection is interleaved into head 0's tk loop to
     fill the ACT-bound window with PE work.
  C) out = x_all-slices.T @ Wo (bf16) -> (T,1024) f32 partial, DMA out
     (alternating sync/gpsimd queues to halve the tail).

PSUM (8 banks): pp 2 (proj/v/out + B-phase x-transpose quads), sc 2x2
(scores [P,1024]; also hosts phase-A transpose quads - disjoint phases),
xps 2 (attnV accumulators). start=True zeroes a whole 2KB bank, so
multi-region accumulation uses start only on the globally-first matmul and
stop on the last; skip_group_check silences the checker for interior ones.

walrus allows only ONE sync-wait per instruction (_split_excess_waits
patches the BIR).
"""

import numpy as np
import ml_dtypes

T = 1024
D = 1024
F = 512      # features per core (8 heads x 64)
NH = 8       # heads per core
DK = 64
P = 128
EPS = 1e-5
BF16 = ml_dtypes.bfloat16

_CACHE = {}

# tuning knobs
PP_BUFS = 2
SC_BUFS = 2
XPS_BUFS = 2
AT_BUFS = 11
XT_BUFS = 2
DRAIN_BUFS = 3
QH_BUFS = 3
STAT_BUFS = 11
OUT_BUFS = 6
DEFER_PAIR_TRANSPOSE = True  # emit pair transposes after next head's first scores


def _split_excess_waits(bj):
    """Walrus allows at most 1 sync-wait per instruction (2 for
    EventSemaphore). Tile's sem assigner can emit more; spill the excess
    onto NoOp carriers inserted just before, on the same engine."""
    import json
    d = json.loads(bj)
    ctr = 0
    for fn in d["functions"]:
        for bb in fn["blocks"]:
            new = []
            for inst in bb["instructions"]:
                si = inst.get("sync_info") or {}
                ow = si.get("on_wait") or []
                op = inst.get("opcode", "")
                cap = 2 if op == "EventSemaphore" else 1
                if len(ow) > cap:
                    for w in ow[:-cap]:
                        ctr += 1
                        new.append({
                            "debug": inst.get("debug", 0),
                            "engine": inst["engine"],
                            "ins": [], "outs": [],
                            "name": f"W-{ctr}",
                            "opcode": "NoOp",
                            "sync_info": {"on_update": [], "on_wait": [w]},
                            "text_hint": "waitsplit",
                        })
                    si["on_wait"] = ow[-cap:]
                new.append(inst)
            bb["instructions"] = new
    return json.dumps(d).encode(), ctr


def _build(use_bq, use_bk, use_bv, ln_beta_zero=True):
    import concourse.bass as bass
    import concourse.tile as tile
    from concourse import mybir
    from concourse.masks import make_identity

    f32 = mybir.dt.float32
    bf16 = mybir.dt.bfloat16
    ALU = mybir.AluOpType
    ACTF = mybir.ActivationFunctionType

    nc = bass.Bass()

    # ---- DRAM I/O ----
    xq_d = nc.dram_tensor("xq16", (D, T), bf16, kind="ExternalInput").ap()
    xk_d = nc.dram_tensor("xk16", (D, T), bf16, kind="ExternalInput").ap()
    xv_d = nc.dram_tensor("xv16", (D, T), bf16, kind="ExternalInput").ap()
    wq_d = nc.dram_tensor("wq16", (D, F), bf16, kind="ExternalInput").ap()
    wk_d = nc.dram_tensor("wk16", (D, F), bf16, kind="ExternalInput").ap()
    wv_d = nc.dram_tensor("wv16", (D, F), bf16, kind="ExternalInput").ap()
    wo_d = nc.dram_tensor("wo16", (F, D), bf16, kind="ExternalInput").ap()
    mask_d = nc.dram_tensor("mask16", (T, T), bf16, kind="ExternalInput").ap()
    # per-partition LN constants (128,1) = per pair-local feature
    gl_d = {}
    for nm in ("gq", "gk"):
        gl_d[nm] = nc.dram_tensor(nm, (P, 1), f32, kind="ExternalInput").ap()
    if not ln_beta_zero:
        for nm in ("bq_ln", "bk_ln"):
            gl_d[nm] = nc.dram_tensor(nm, (P, 1), f32, kind="ExternalInput").ap()
    biases = {}
    for name, used in (("bq", use_bq), ("bk", use_bk), ("bv", use_bv)):
        if used:
            biases[name] = nc.dram_tensor(name, (F,), f32, kind="ExternalInput").ap()
    out_p = nc.dram_tensor("out_p", (T, D), f32, kind="ExternalOutput").ap()

    # DRAM views
    xviews = {
        "q": xq_d.rearrange("(dc p) t -> p dc t", p=P),
        "k": xk_d.rearrange("(dc p) t -> p dc t", p=P),
        "v": xv_d.rearrange("(dc p) t -> p dc t", p=P),
    }
    wviews = {
        "q": wq_d.rearrange("(dc p) f -> p dc f", p=P),
        "k": wk_d.rearrange("(dc p) f -> p dc f", p=P),
        "v": wv_d.rearrange("(dc p) f -> p dc f", p=P),
    }
    wo_view = wo_d.rearrange("(fc p) d -> p fc d", p=P)
    mask_view = mask_d.rearrange("(kc p) t -> p kc t", p=P)
    out_view = out_p.rearrange("(tc p) d -> p tc d", p=P)

    with tile.TileContext(nc) as tc:
        with (
            tc.tile_pool(name="const", bufs=1) as const,
            tc.tile_pool(name="drain", bufs=DRAIN_BUFS) as drain,
            tc.tile_pool(name="stat", bufs=STAT_BUFS) as stat,
            tc.tile_pool(name="qhatp", bufs=QH_BUFS) as qhatp,
            tc.tile_pool(name="attnp", bufs=AT_BUFS) as attnp,
            tc.tile_pool(name="xtp", bufs=XT_BUFS) as xtp,
            tc.tile_pool(name="recipp", bufs=4) as recipp,
            tc.tile_pool(name="outp", bufs=OUT_BUFS) as outp,
            tc.tile_pool(name="psum", bufs=1, space="PSUM") as psum,
        ):
            def pp_tile(shape=(P, F), dtype=f32, name="pp"):
                return psum.tile(list(shape), dtype, name=name, tag="pp",
                                 bufs=PP_BUFS)

            def sc_tile(shape=(P, T), dtype=f32, name="sc"):
                return psum.tile(list(shape), dtype, name=name, tag="sc",
                                 bufs=SC_BUFS)

            def xps_tile(g):
                return psum.tile([P, 4, P], f32, name=f"xps{g}", tag="xps",
                                 bufs=XPS_BUFS)

            # ---- resident tiles ----
            x_sb = {
                pn: const.tile([P, 8, T], bf16, name=f"x{pn}_sb", tag=f"x{pn}_sb")
                for pn in ("q", "k", "v")
            }
            w_sb = {
                pn: const.tile([P, 8, F], bf16, name=f"w{pn}_sb", tag=f"w{pn}_sb")
                for pn in ("q", "k", "v")
            }
            wo_sb = const.tile([P, 4, D], bf16, name="wo", tag="wo")
            mask_sb = const.tile([P, 8, T], bf16, name="mask", tag="mask")
            qlnT = const.tile([P, 4, T], bf16, name="qlnT", tag="qlnT")
            klnT = const.tile([P, 4, T], bf16, name="klnT", tag="klnT")
            v_sb = const.tile([P, 8, NH, 65], bf16, name="v_sb", tag="v_sb")
            x_all = const.tile([P, 4, T], bf16, name="x_all", tag="x_all")
            eps_t = const.tile([P, 1], f32, name="eps", tag="eps")
            gb_t = {}
            for nm, dr_ in gl_d.items():
                gb_t[nm] = const.tile([P, 1], f32, name=f"ln_{nm}", tag=f"ln_{nm}")
                nc.gpsimd.dma_start(gb_t[nm], dr_)
            nc.vector.memset(eps_t, EPS)
            ident16 = const.tile([P, P], bf16, name="ident16", tag="ident16")
            make_identity(nc, ident16)
            # ones column for softmax denominators
            nc.vector.memset(v_sb[:, :, :, 64:65], 1.0)


            bias_bc = {}
            for name in biases:
                bias_bc[name] = const.tile([P, F], f32, name=f"bc_{name}", tag=f"bc_{name}")
                src = bass.AP(
                    tensor=biases[name].tensor,
                    offset=biases[name].offset,
                    ap=[[0, P], [1, F]],
                )
                nc.gpsimd.dma_start(out=bias_bc[name], in_=src)

            # ---- input DMAs ----
            # sync queue, ordered so the k-projection can start ASAP
            def load_x(pn, quarters):
                for qtr in quarters:
                    sl = slice(qtr * 256, (qtr + 1) * 256)
                    nc.sync.dma_start(x_sb[pn][:, :, sl], xviews[pn][:, :, sl])

            nc.sync.dma_start(w_sb["k"][:, 0:4, :], wviews["k"][:, 0:4, :])
            load_x("k", range(1))
            nc.sync.dma_start(w_sb["k"][:, 4:8, :], wviews["k"][:, 4:8, :])
            load_x("k", range(1, 4))
            nc.sync.dma_start(w_sb["q"], wviews["q"])
            load_x("q", range(2))
            nc.sync.dma_start(w_sb["v"], wviews["v"])
            load_x("q", range(2, 4))
            load_x("v", range(4))
            nc.sync.dma_start(wo_sb, wo_view)
            # mask via the SWDGE queue, concurrent with the sync queue
            for half in range(2):
                nc.gpsimd.dma_start(mask_sb[:, 4 * half:4 * half + 4, :],
                                    mask_view[:, 4 * half:4 * half + 4, :])

            # ---- Phase A: q/k projections + LN + transpose ----
            a_pending = []

            def flush_a():
                while a_pending:
                    a_pending.pop(0)()

            def proj_ln(pn, dstT, t):
                bias_name = "b" + pn
                ps = pp_tile()
                for d in range(8):
                    nc.tensor.matmul(
                        ps, lhsT=x_sb[pn][:, d, t * P:(t + 1) * P],
                        rhs=w_sb[pn][:, d, :],
                        start=(d == 0), stop=(d == 7),
                    )
                # deferred transposes/gamma-drains of older tiles go here:
                # after this tile's matmuls (PE) and before its drain (ACT),
                # so neither engine's in-order queue blocks them.
                while len(a_pending) > 3:
                    a_pending.pop(0)()
                sb = drain.tile([P, NH, DK], f32, name="qsb", tag="qsb")
                if bias_name in bias_bc:
                    nc.vector.tensor_add(
                        sb.rearrange("p h d -> p (h d)"), ps, bias_bc[bias_name])
                else:
                    nc.scalar.activation(
                        out=sb.rearrange("p h d -> p (h d)"), in_=ps,
                        func=ACTF.Copy)
                st = stat.tile([P, NH, 6], f32, name="st", tag="st")
                for h in range(NH):
                    nc.vector.bn_stats(out=st[:, h, :], in_=sb[:, h, :])
                ag = stat.tile([P, NH, 2], f32, name="ag", tag="ag")
                for h in range(NH):
                    nc.vector.bn_aggr(out=ag[:, h, :], in_=st[:, h, :])
                sd = stat.tile([P, NH], f32, name="sd", tag="sd")
                nc.scalar.activation(
                    out=sd, in_=ag[:, :, 1], func=ACTF.Sqrt, bias=eps_t)
                rs = stat.tile([P, NH], f32, name="rs", tag="rs")
                nc.vector.reciprocal(out=rs, in_=sd)
                qh = qhatp.tile([P, F], bf16, name="qh", tag="qh")
                for h in range(NH):
                    nc.gpsimd.tensor_scalar(
                        out=qh[:, h * DK:(h + 1) * DK],
                        in0=sb[:, h, :],
                        scalar1=ag[:, h, 0:1],
                        scalar2=rs[:, h:h + 1],
                        op0=ALU.subtract,
                        op1=ALU.mult,
                    )
                # 4 pair-transposes into one [P,512] PSUM quad (on the sc
                # tag: scores don't run during phase A), then ONE gamma
                # scale-drain. Deferred one tile so the PE doesn't wait on
                # the LN chain.
                def emit(qh=qh, pn=pn, dstT=dstT, t=t):
                    pst4 = sc_tile((P, 4, P), bf16, name="pst4")
                    for j in range(4):
                        nc.tensor.transpose(
                            pst4[:, j, :], qh[:, j * P:(j + 1) * P], ident16)
                    g_nm, b_nm = ("gq", "bq_ln") if pn == "q" else ("gk", "bk_ln")
                    dst = dstT[:, :, t * P:(t + 1) * P]
                    if ln_beta_zero:
                        nc.scalar.activation(
                            out=dst, in_=pst4, func=ACTF.Copy, scale=gb_t[g_nm])
                    else:
                        nc.scalar.tensor_scalar(
                            out=dst, in0=pst4,
                            scalar1=gb_t[g_nm], scalar2=gb_t[b_nm],
                            op0=ALU.mult, op1=ALU.add)
                a_pending.append(emit)

            order = [("k", klnT, t) for t in range(8)] + \
                [("q", qlnT, t) for t in range(8)]
            for pn, dstT, t in order:
                proj_ln(pn, dstT, t)
            while a_pending:
                a_pending.pop(0)()

            # ---- Phase B: attention, one flat software pipeline ----
            # Per unit (h, tk): emit scores/exp/mask, then the PREVIOUS
            # unit's attnV matmuls, so the PE never waits on exp+mask.
            # Head drains (reciprocal + scale) and pair transposes are
            # emitted when that head's last attnV retires.
            pending = []  # deferred pair-transpose emitters

            def flush_pending():
                while pending:
                    pending.pop(0)()

            xps_h = {}
            xTb_h = {}

            def attn_v(h, tk, at):
                xps = xps_h[h]
                for qg in range(2):
                    for qb in range(4):
                        j = qg * 4 + qb
                        first = (tk == 0 and qb == 0)
                        last = (tk == 7 and qb == 3)
                        nc.tensor.matmul(
                            xps[qg][:, qb, 0:65],
                            lhsT=at[:, j * P:(j + 1) * P],
                            rhs=v_sb[:, tk, h, :],
                            start=first, stop=last,
                            skip_group_check=not (first or last),
                        )

            def head_drain(h):
                xps = xps_h.pop(h)
                if h % 2 == 0:
                    xTb_h[h // 2] = xtp.tile([P, 8, P], bf16, name="xTb", tag="xTb")
                xTb = xTb_h[h // 2]
                csl = slice(0, DK) if h % 2 == 0 else slice(DK, P)
                for qg in range(2):
                    rc = recipp.tile([P, 4], f32, name="rc", tag="rc")
                    nc.vector.reciprocal(out=rc, in_=xps[qg][:, :, 64:65])
                    for qb in range(4):
                        nc.vector.tensor_scalar(
                            out=xTb[:, qg * 4 + qb, csl],
                            in0=xps[qg][:, qb, 0:64],
                            scalar1=rc[:, qb:qb + 1], scalar2=None,
                            op0=ALU.mult)
                if h % 2 == 1:
                    jj = h // 2

                    def emit_transposes(xTb=xTb, jj=jj):
                        for qg in range(2):
                            pst4 = pp_tile((P, 4, P), bf16, name="pstx")
                            for qb in range(4):
                                nc.tensor.transpose(
                                    pst4[:, qb, :], xTb[:, qg * 4 + qb, :], ident16)
                            nc.vector.tensor_copy(
                                out=x_all[:, jj, qg * F:(qg + 1) * F],
                                in_=pst4.rearrange("p a b -> p (a b)"))
                    if DEFER_PAIR_TRANSPOSE and h < NH - 1:
                        pending.append(emit_transposes)
                    else:
                        emit_transposes()

            at_q = []
            v_tasks = []  # (tk, d_lo, d_hi, drain?) chunks, 4 matmuls each
            for tk in range(8):
                v_tasks.append((tk, 0, 4, False))
                v_tasks.append((tk, 4, 8, True))
            v_ps = {}

            def v_chunk():
                tk, dlo, dhi, do_drain = v_tasks.pop(0)
                if dlo == 0:
                    v_ps[tk] = pp_tile()
                ps = v_ps[tk]
                for d in range(dlo, dhi):
                    nc.tensor.matmul(
                        ps, lhsT=x_sb["v"][:, d, tk * P:(tk + 1) * P],
                        rhs=w_sb["v"][:, d, :],
                        start=(d == 0), stop=(d == 7),
                    )
                if do_drain:
                    ps = v_ps.pop(tk)
                    if "bv" in bias_bc:
                        vb = drain.tile([P, NH, DK], f32, name="vsb", tag="qsb")
                        nc.vector.tensor_add(
                            vb.rearrange("p h d -> p (h d)"), ps, bias_bc["bv"])
                        nc.gpsimd.tensor_copy(out=v_sb[:, tk, :, 0:64], in_=vb)
                    else:
                        nc.vector.tensor_copy(
                            out=v_sb[:, tk, :, 0:64],
                            in_=ps.rearrange("p (h c) -> p h c", c=DK))

            def pop_attnv():
                hp, tkp, atp_ = at_q.pop(0)
                attn_v(hp, tkp, atp_)
                if tkp == 7:
                    head_drain(hp)
                if hp % 2 == 1 and tkp == 1:
                    flush_pending()

            u = 0
            for h in range(NH):
                rows = slice((h % 2) * DK, (h % 2) * DK + DK)
                pair = h // 2
                xps_h[h] = [xps_tile(g) for g in range(2)]
                for tk in range(8):
                    # v-projection interleaved into head 0's units; v(tk)
                    # must be emitted before attnV(0,tk) is popped (Tile
                    # orders only already-emitted instructions)
                    if h == 0:
                        v_chunk()
                        v_chunk()
                    sp = sc_tile()
                    for n in range(2):
                        nc.tensor.matmul(
                            sp[:, n * F:(n + 1) * F],
                            lhsT=klnT[rows, pair, tk * P:(tk + 1) * P],
                            rhs=qlnT[rows, pair, n * F:(n + 1) * F],
                            start=True, stop=True,
                        )
                    at = attnp.tile([P, T], bf16, name="at", tag="at")
                    nc.scalar.activation(out=at, in_=sp, func=ACTF.Exp)
                    nc.vector.tensor_mul(at, at, mask_sb[:, tk, :])
                    at_q.append((h, tk, at))
                    while len(at_q) > 1:
                        pop_attnv()
                    u += 1
            while at_q:
                pop_attnv()
            flush_pending()

            # ---- Phase C: output projection ----
            for t in range(8):
                for n in range(2):
                    r = (2 * t + n) % 3
                    if r == 0:
                        ps = pp_tile()
                    elif r == 1:
                        ps = sc_tile((P, F), f32, name="scc")
                    else:
                        ps = psum.tile([P, F], f32, name="xpc", tag="xps",
                                       bufs=XPS_BUFS)
                    for jj in range(4):
                        nc.tensor.matmul(
                            ps, lhsT=x_all[:, jj, t * P:(t + 1) * P],
                            rhs=wo_sb[:, jj, n * F:(n + 1) * F],
                            start=(jj == 0), stop=(jj == 3),
                        )
                    ob = outp.tile([P, F], f32, name="ob", tag="ob")
                    nc.vector.tensor_copy(out=ob, in_=ps)
                    q_eng = nc.sync if n == 0 else nc.scalar
                    q_eng.dma_start(out=out_view[:, t, n * F:(n + 1) * F], in_=ob)

    return nc


def _get_nc(flags):
    if len(flags) == 3:
        flags = (*flags, True)
    key = flags
    if key not in _CACHE:
        nc = _build(*flags)
        patched, _n = _split_excess_waits(nc.to_json_bytes())
        nc.to_json_bytes = lambda: patched
        _CACHE[key] = nc
    return _CACHE[key]


def _bf(a):
    return np.ascontiguousarray(np.asarray(a).astype(BF16))


def kernel(query, key, value, mask, Wq, bq, Wk, bk, Wv, bv, Wo, bo,
           q_gamma, q_beta, k_gamma, k_beta, _trace=False):
    from concourse.bass_utils import run_bass_kernel_spmd

    query = np.asarray(query, np.float32)
    key = np.asarray(key, np.float32)
    value = np.asarray(value, np.float32)
    mask = np.asarray(mask)
    Wq, Wk, Wv, Wo = (np.asarray(w, np.float32) for w in (Wq, Wk, Wv, Wo))
    bq, bk, bv, bo = (np.asarray(b, np.float32) for b in (bq, bk, bv, bo))
    q_gamma, q_beta, k_gamma, k_beta = (
        np.asarray(g, np.float32) for g in (q_gamma, q_beta, k_gamma, k_beta))

    B = query.shape[0]
    use_bq, use_bk, use_bv = (bool(np.any(b)) for b in (bq, bk, bv))
    ln_beta_zero = not (np.any(q_beta) or np.any(k_beta))
    nc = _get_nc((use_bq, use_bk, use_bv, ln_beta_zero))

    # host-side shard prep
    xq16 = [_bf(query[b].T) for b in range(B)]
    xk16 = [_bf(key[b].T) for b in range(B)]
    xv16 = [_bf(value[b].T) for b in range(B)]
    mask16 = [np.ascontiguousarray((~mask[b]).T.astype(BF16)) for b in range(B)]
    # per-partition LN consts (pair-local feature); q folds 1/sqrt(dk)=1/8
    def tile2(v):
        return np.ascontiguousarray(np.tile(v, 2).reshape(P, 1).astype(np.float32))
    consts = {
        "gq": tile2(q_gamma / 8.0),
        "gk": tile2(k_gamma),
    }
    if not ln_beta_zero:
        consts.update({
            "bq_ln": tile2(q_beta / 8.0),
            "bk_ln": tile2(k_beta),
        })

    in_maps = []
    for core in range(8):
        b, g = core // 2, core % 2
        sl = slice(g * F, (g + 1) * F)
        im = {
            "xq16": xq16[b], "xk16": xk16[b], "xv16": xv16[b],
            "wq16": _bf(Wq[sl].T),
            "wk16": _bf(Wk[sl].T),
            "wv16": _bf(Wv[sl].T),
            "wo16": _bf(Wo[:, sl].T),
            "mask16": mask16[b],
            **consts,
        }
        if use_bq:
            im["bq"] = np.ascontiguousarray(bq[sl])
        if use_bk:
            im["bk"] = np.ascontiguousarray(bk[sl])
        if use_bv:
            im["bv"] = np.ascontiguousarray(bv[sl])
        in_maps.append(im)

    res = run_bass_kernel_spmd(nc, in_maps, core_ids=list(range(8)), trace=_trace)
    out = np.zeros((B, T, D), np.float32)
    for b in range(B):
        out[b] = res.results[2 * b]["out_p"] + res.results[2 * b + 1]["out_p"] + bo
    if _trace:
        kernel._last_results = res
    return out


# revision 38
# speedup vs baseline: 1.4321x; 1.0286x over previous
"""Trainium2 Bass kernel for nn_MultiHeadAttention (B=4, T=1024, D=1024, H=16, dk=64).

Sharding: 8 cores = 4 batches x 2 head-groups (8 heads / 512 features each).
Each core computes a partial output (its head-group's contribution through Wo);
host sums the two partials per batch and adds bo.

Per-core dataflow (one NeuronCore, Tile-scheduled), cost-model-guided; all
matmuls bf16 (fp8 DoubleRow was tried and rejected: e4m3's 2.65% RMS lands
~6% on the output, over the 2e-2 gate):
  A) q/k projections (bf16, 8 d-chunk accumulation) -> LN per head
     (bn_stats + bn_aggr, Rsqrt(var+eps) on ACT) -> normalize (Pool) ->
     qh bf16 -> 4 PE transposes per tile into a [P,512] PSUM quad ->
     one ACT scale-drain (gamma, and 1/sqrt(dk) on q) into qlnT/klnT
     [pair-features x 4 pairs x T].
  B) head-major attention: scoresT per (h, tk-block) = klnT-slice.T @ qlnT
     (2x 512-chunk matmuls, K=64) -> PSUM [128,1024]; exp on ACT (the
     bottleneck engine of this phase - nothing else is scheduled on ACT
     here); mask multiply on DVE; attnV transposed: out partitions = query
     positions, rhs = v_sb[tk,h,:] = [v|1] (65 cols, ones column yields
     softmax denominators in col 64) accumulated over tk chunks into
     bank-sized PSUM [128,4,128] tiles; DVE reciprocal + per-qb
     tensor_scalar -> xT bf16; after each odd head 8 PE transposes ->
     x_all [f, T]. v-projection is interleaved into head 0's tk loop to
     fill the ACT-bound window with PE work.
  C) out = x_all-slices.T @ Wo (bf16) -> (T,1024) f32 partial, DMA out
     (alternating sync/gpsimd queues to halve the tail).

PSUM (8 banks): pp 2 (proj/v/out + B-phase x-transpose quads), sc 2x2
(scores [P,1024]; also hosts phase-A transpose quads - disjoint phases),
xps 2 (attnV accumulators). start=True zeroes a whole 2KB bank, so
multi-region accumulation uses start only on the globally-first matmul and
stop on the last; skip_group_check silences the checker for interior ones.

walrus allows only ONE sync-wait per instruction (_split_excess_waits
patches the BIR).
"""

import numpy as np
import ml_dtypes

T = 1024
D = 1024
F = 512      # features per core (8 heads x 64)
NH = 8       # heads per core
DK = 64
P = 128
EPS = 1e-5
BF16 = ml_dtypes.bfloat16

_CACHE = {}

# tuning knobs
PP_BUFS = 2
SC_BUFS = 2
XPS_BUFS = 2
AT_BUFS = 11
XT_BUFS = 2
DRAIN_BUFS = 3
QH_BUFS = 3
STAT_BUFS = 11
OUT_BUFS = 6
DEFER_PAIR_TRANSPOSE = True  # emit pair transposes after next head's first scores


def _split_excess_waits(bj):
    """Walrus allows at most 1 sync-wait per instruction (2 for
    EventSemaphore). Tile's sem assigner can emit more; spill the excess
    onto NoOp carriers inserted just before, on the same engine."""
    import json
    d = json.loads(bj)
    ctr = 0
    for fn in d["functions"]:
        for bb in fn["blocks"]:
            new = []
            for inst in bb["instructions"]:
                si = inst.get("sync_info") or {}
                ow = si.get("on_wait") or []
                op = inst.get("opcode", "")
                cap = 2 if op == "EventSemaphore" else 1
                if len(ow) > cap:
                    for w in ow[:-cap]:
                        ctr += 1
                        new.append({
                            "debug": inst.get("debug", 0),
                            "engine": inst["engine"],
                            "ins": [], "outs": [],
                            "name": f"W-{ctr}",
                            "opcode": "NoOp",
                            "sync_info": {"on_update": [], "on_wait": [w]},
                            "text_hint": "waitsplit",
                        })
                    si["on_wait"] = ow[-cap:]
                new.append(inst)
            bb["instructions"] = new
    return json.dumps(d).encode(), ctr


def _build(use_bq, use_bk, use_bv, ln_beta_zero=True):
    import concourse.bass as bass
    import concourse.tile as tile
    from concourse import mybir
    from concourse.masks import make_identity

    f32 = mybir.dt.float32
    bf16 = mybir.dt.bfloat16
    ALU = mybir.AluOpType
    ACTF = mybir.ActivationFunctionType

    nc = bass.Bass()

    # ---- DRAM I/O ----
    xq_d = nc.dram_tensor("xq16", (D, T), bf16, kind="ExternalInput").ap()
    xk_d = nc.dram_tensor("xk16", (D, T), bf16, kind="ExternalInput").ap()
    xv_d = nc.dram_tensor("xv16", (D, T), bf16, kind="ExternalInput").ap()
    wq_d = nc.dram_tensor("wq16", (D, F), bf16, kind="ExternalInput").ap()
    wk_d = nc.dram_tensor("wk16", (D, F), bf16, kind="ExternalInput").ap()
    wv_d = nc.dram_tensor("wv16", (D, F), bf16, kind="ExternalInput").ap()
    wo_d = nc.dram_tensor("wo16", (F, D), bf16, kind="ExternalInput").ap()
    mask_d = nc.dram_tensor("mask16", (T, T), bf16, kind="ExternalInput").ap()
    # per-partition LN constants (128,1) = per pair-local feature
    gl_d = {}
    for nm in ("gq", "gk"):
        gl_d[nm] = nc.dram_tensor(nm, (P, 1), f32, kind="ExternalInput").ap()
    if not ln_beta_zero:
        for nm in ("bq_ln", "bk_ln"):
            gl_d[nm] = nc.dram_tensor(nm, (P, 1), f32, kind="ExternalInput").ap()
    biases = {}
    for name, used in (("bq", use_bq), ("bk", use_bk), ("bv", use_bv)):
        if used:
            biases[name] = nc.dram_tensor(name, (F,), f32, kind="ExternalInput").ap()
    out_p = nc.dram_tensor("out_p", (T, D), bf16, kind="ExternalOutput").ap()

    # DRAM views
    xviews = {
        "q": xq_d.rearrange("(dc p) t -> p dc t", p=P),
        "k": xk_d.rearrange("(dc p) t -> p dc t", p=P),
        "v": xv_d.rearrange("(dc p) t -> p dc t", p=P),
    }
    wviews = {
        "q": wq_d.rearrange("(dc p) f -> p dc f", p=P),
        "k": wk_d.rearrange("(dc p) f -> p dc f", p=P),
        "v": wv_d.rearrange("(dc p) f -> p dc f", p=P),
    }
    wo_view = wo_d.rearrange("(fc p) d -> p fc d", p=P)
    mask_view = mask_d.rearrange("(kc p) t -> p kc t", p=P)
    out_view = out_p.rearrange("(tc p) d -> p tc d", p=P)

    with tile.TileContext(nc) as tc:
        with (
            tc.tile_pool(name="const", bufs=1) as const,
            tc.tile_pool(name="drain", bufs=DRAIN_BUFS) as drain,
            tc.tile_pool(name="stat", bufs=STAT_BUFS) as stat,
            tc.tile_pool(name="qhatp", bufs=QH_BUFS) as qhatp,
            tc.tile_pool(name="attnp", bufs=AT_BUFS) as attnp,
            tc.tile_pool(name="xtp", bufs=XT_BUFS) as xtp,
            tc.tile_pool(name="recipp", bufs=4) as recipp,
            tc.tile_pool(name="outp", bufs=OUT_BUFS) as outp,
            tc.tile_pool(name="psum", bufs=1, space="PSUM") as psum,
        ):
            def pp_tile(shape=(P, F), dtype=f32, name="pp"):
                return psum.tile(list(shape), dtype, name=name, tag="pp",
                                 bufs=PP_BUFS)

            def sc_tile(shape=(P, T), dtype=f32, name="sc"):
                return psum.tile(list(shape), dtype, name=name, tag="sc",
                                 bufs=SC_BUFS)

            def xps_tile(g):
                return psum.tile([P, 4, P], f32, name=f"xps{g}", tag="xps",
                                 bufs=XPS_BUFS)

            # ---- resident tiles ----
            x_sb = {
                pn: const.tile([P, 8, T], bf16, name=f"x{pn}_sb", tag=f"x{pn}_sb")
                for pn in ("q", "k", "v")
            }
            w_sb = {
                pn: const.tile([P, 8, F], bf16, name=f"w{pn}_sb", tag=f"w{pn}_sb")
                for pn in ("q", "k", "v")
            }
            wo_sb = const.tile([P, 4, D], bf16, name="wo", tag="wo")
            mask_sb = const.tile([P, 8, T], bf16, name="mask", tag="mask")
            qlnT = const.tile([P, 4, T], bf16, name="qlnT", tag="qlnT")
            klnT = const.tile([P, 4, T], bf16, name="klnT", tag="klnT")
            v_sb = const.tile([P, 8, NH, 65], bf16, name="v_sb", tag="v_sb")
            x_all = const.tile([P, 4, T], bf16, name="x_all", tag="x_all")
            eps_t = const.tile([P, 1], f32, name="eps", tag="eps")
            gb_t = {}
            for nm, dr_ in gl_d.items():
                gb_t[nm] = const.tile([P, 1], f32, name=f"ln_{nm}", tag=f"ln_{nm}")
                nc.gpsimd.dma_start(gb_t[nm], dr_)
            nc.vector.memset(eps_t, EPS)
            ident16 = const.tile([P, P], bf16, name="ident16", tag="ident16")
            make_identity(nc, ident16)
            # ones column for softmax denominators
            nc.vector.memset(v_sb[:, :, :, 64:65], 1.0)


            bias_bc = {}
            for name in biases:
                bias_bc[name] = const.tile([P, F], f32, name=f"bc_{name}", tag=f"bc_{name}")
                src = bass.AP(
                    tensor=biases[name].tensor,
                    offset=biases[name].offset,
                    ap=[[0, P], [1, F]],
                )
                nc.gpsimd.dma_start(out=bias_bc[name], in_=src)

            # ---- input DMAs ----
            # sync queue, ordered so the k-projection can start ASAP
            def load_x(pn, quarters):
                for qtr in quarters:
                    sl = slice(qtr * 256, (qtr + 1) * 256)
                    nc.sync.dma_start(x_sb[pn][:, :, sl], xviews[pn][:, :, sl])

            nc.sync.dma_start(w_sb["k"][:, 0:1, :], wviews["k"][:, 0:1, :])
            load_x("k", range(1))
            for d in range(1, 8):
                nc.sync.dma_start(w_sb["k"][:, d:d + 1, :], wviews["k"][:, d:d + 1, :])
            load_x("k", range(1, 4))
            nc.sync.dma_start(w_sb["q"], wviews["q"])
            load_x("q", range(2))
            nc.sync.dma_start(w_sb["v"], wviews["v"])
            load_x("q", range(2, 4))
            load_x("v", range(4))
            nc.sync.dma_start(wo_sb, wo_view)
            # mask via the SWDGE queue, concurrent with the sync queue
            for half in range(2):
                nc.gpsimd.dma_start(mask_sb[:, 4 * half:4 * half + 4, :],
                                    mask_view[:, 4 * half:4 * half + 4, :])

            # ---- Phase A: q/k projections + LN + transpose ----
            a_pending = []

            def flush_a():
                while a_pending:
                    a_pending.pop(0)()

            def proj_ln(pn, dstT, t):
                bias_name = "b" + pn
                ps = pp_tile()
                for d in range(8):
                    nc.tensor.matmul(
                        ps, lhsT=x_sb[pn][:, d, t * P:(t + 1) * P],
                        rhs=w_sb[pn][:, d, :],
                        start=(d == 0), stop=(d == 7),
                    )
                # deferred transposes/gamma-drains of older tiles go here:
                # after this tile's matmuls (PE) and before its drain (ACT),
                # so neither engine's in-order queue blocks them.
                while len(a_pending) > 3:
                    a_pending.pop(0)()
                sb = drain.tile([P, NH, DK], f32, name="qsb", tag="qsb")
                if bias_name in bias_bc:
                    nc.vector.tensor_add(
                        sb.rearrange("p h d -> p (h d)"), ps, bias_bc[bias_name])
                else:
                    nc.scalar.activation(
                        out=sb.rearrange("p h d -> p (h d)"), in_=ps,
                        func=ACTF.Copy)
                st = stat.tile([P, NH, 6], f32, name="st", tag="st")
                for h in range(NH):
                    nc.vector.bn_stats(out=st[:, h, :], in_=sb[:, h, :])
                ag = stat.tile([P, NH, 2], f32, name="ag", tag="ag")
                for h in range(NH):
                    nc.vector.bn_aggr(out=ag[:, h, :], in_=st[:, h, :])
                sd = stat.tile([P, NH], f32, name="sd", tag="sd")
                nc.scalar.activation(
                    out=sd, in_=ag[:, :, 1], func=ACTF.Sqrt, bias=eps_t)
                rs = stat.tile([P, NH], f32, name="rs", tag="rs")
                nc.vector.reciprocal(out=rs, in_=sd)
                qh = qhatp.tile([P, F], bf16, name="qh", tag="qh")
                for h in range(NH):
                    nc.gpsimd.tensor_scalar(
                        out=qh[:, h * DK:(h + 1) * DK],
                        in0=sb[:, h, :],
                        scalar1=ag[:, h, 0:1],
                        scalar2=rs[:, h:h + 1],
                        op0=ALU.subtract,
                        op1=ALU.mult,
                    )
                # 4 pair-transposes into one [P,512] PSUM quad (on the sc
                # tag: scores don't run during phase A), then ONE gamma
                # scale-drain. Deferred one tile so the PE doesn't wait on
                # the LN chain.
                def emit(qh=qh, pn=pn, dstT=dstT, t=t):
                    pst4 = sc_tile((P, 4, P), bf16, name="pst4")
                    for j in range(4):
                        nc.tensor.transpose(
                            pst4[:, j, :], qh[:, j * P:(j + 1) * P], ident16)
                    g_nm, b_nm = ("gq", "bq_ln") if pn == "q" else ("gk", "bk_ln")
                    dst = dstT[:, :, t * P:(t + 1) * P]
                    if ln_beta_zero:
                        nc.scalar.activation(
                            out=dst, in_=pst4, func=ACTF.Copy, scale=gb_t[g_nm])
                    else:
                        nc.scalar.tensor_scalar(
                            out=dst, in0=pst4,
                            scalar1=gb_t[g_nm], scalar2=gb_t[b_nm],
                            op0=ALU.mult, op1=ALU.add)
                a_pending.append(emit)

            order = [("k", klnT, t) for t in range(8)] + \
                [("q", qlnT, t) for t in range(8)]
            for pn, dstT, t in order:
                proj_ln(pn, dstT, t)
            while a_pending:
                a_pending.pop(0)()

            # ---- Phase B: attention, one flat software pipeline ----
            # Per unit (h, tk): emit scores/exp/mask, then the PREVIOUS
            # unit's attnV matmuls, so the PE never waits on exp+mask.
            # Head drains (reciprocal + scale) and pair transposes are
            # emitted when that head's last attnV retires.
            pending = []  # deferred pair-transpose emitters

            def flush_pending():
                while pending:
                    pending.pop(0)()

            xps_h = {}
            xTb_h = {}

            def attn_v(h, tk, at):
                xps = xps_h[h]
                for qg in range(2):
                    for qb in range(4):
                        j = qg * 4 + qb
                        first = (tk == 0 and qb == 0)
                        last = (tk == 7 and qb == 3)
                        nc.tensor.matmul(
                            xps[qg][:, qb, 0:65],
                            lhsT=at[:, j * P:(j + 1) * P],
                            rhs=v_sb[:, tk, h, :],
                            start=first, stop=last,
                            skip_group_check=not (first or last),
                        )

            def head_drain(h):
                xps = xps_h.pop(h)
                if h % 2 == 0:
                    xTb_h[h // 2] = xtp.tile([P, 8, P], bf16, name="xTb", tag="xTb")
                xTb = xTb_h[h // 2]
                csl = slice(0, DK) if h % 2 == 0 else slice(DK, P)
                for qg in range(2):
                    rc = recipp.tile([P, 4], f32, name="rc", tag="rc")
                    nc.vector.reciprocal(out=rc, in_=xps[qg][:, :, 64:65])
                    for qb in range(4):
                        nc.vector.tensor_scalar(
                            out=xTb[:, qg * 4 + qb, csl],
                            in0=xps[qg][:, qb, 0:64],
                            scalar1=rc[:, qb:qb + 1], scalar2=None,
                            op0=ALU.mult)
                if h % 2 == 1:
                    jj = h // 2

                    def emit_transposes(xTb=xTb, jj=jj):
                        for qg in range(2):
                            pst4 = pp_tile((P, 4, P), bf16, name="pstx")
                            for qb in range(4):
                                nc.tensor.transpose(
                                    pst4[:, qb, :], xTb[:, qg * 4 + qb, :], ident16)
                            nc.vector.tensor_copy(
                                out=x_all[:, jj, qg * F:(qg + 1) * F],
                                in_=pst4.rearrange("p a b -> p (a b)"))
                    if DEFER_PAIR_TRANSPOSE and h < NH - 1:
                        pending.append(emit_transposes)
                    else:
                        emit_transposes()

            at_q = []
            v_tasks = []  # (tk, d_lo, d_hi, drain?) chunks, 4 matmuls each
            for tk in range(8):
                v_tasks.append((tk, 0, 4, False))
                v_tasks.append((tk, 4, 8, True))
            v_ps = {}

            def v_chunk():
                tk, dlo, dhi, do_drain = v_tasks.pop(0)
                if dlo == 0:
                    v_ps[tk] = pp_tile()
                ps = v_ps[tk]
                for d in range(dlo, dhi):
                    nc.tensor.matmul(
                        ps, lhsT=x_sb["v"][:, d, tk * P:(tk + 1) * P],
                        rhs=w_sb["v"][:, d, :],
                        start=(d == 0), stop=(d == 7),
                    )
                if do_drain:
                    ps = v_ps.pop(tk)
                    if "bv" in bias_bc:
                        vb = drain.tile([P, NH, DK], f32, name="vsb", tag="qsb")
                        nc.vector.tensor_add(
                            vb.rearrange("p h d -> p (h d)"), ps, bias_bc["bv"])
                        nc.gpsimd.tensor_copy(out=v_sb[:, tk, :, 0:64], in_=vb)
                    else:
                        nc.vector.tensor_copy(
                            out=v_sb[:, tk, :, 0:64],
                            in_=ps.rearrange("p (h c) -> p h c", c=DK))

            def pop_attnv():
                hp, tkp, atp_ = at_q.pop(0)
                attn_v(hp, tkp, atp_)
                if tkp == 7:
                    head_drain(hp)
                if hp % 2 == 1 and tkp == 1:
                    flush_pending()

            u = 0
            for h in range(NH):
                rows = slice((h % 2) * DK, (h % 2) * DK + DK)
                pair = h // 2
                xps_h[h] = [xps_tile(g) for g in range(2)]
                for tk in range(8):
                    # v-projection: one 4-matmul chunk per unit over the
                    # first 16 units; h=0's attnV lags 9 units so v(tk) is
                    # always emitted before its consumer is popped
                    if v_tasks:
                        v_chunk()
                    sp = sc_tile()
                    for n in range(2):
                        nc.tensor.matmul(
                            sp[:, n * F:(n + 1) * F],
                            lhsT=klnT[rows, pair, tk * P:(tk + 1) * P],
                            rhs=qlnT[rows, pair, n * F:(n + 1) * F],
                            start=True, stop=True,
                        )
                    at = attnp.tile([P, T], bf16, name="at", tag="at")
                    nc.scalar.activation(out=at, in_=sp, func=ACTF.Exp)
                    nc.vector.tensor_mul(at, at, mask_sb[:, tk, :])
                    at_q.append((h, tk, at))
                    target = 9 if u < 16 else max(1, 9 - (u - 16) // 2)
                    while len(at_q) > target:
                        pop_attnv()
                    u += 1
            while at_q:
                pop_attnv()
            flush_pending()

            # ---- Phase C: output projection ----
            for t in range(8):
                for n in range(2):
                    r = (2 * t + n) % 3
                    if r == 0:
                        ps = pp_tile()
                    elif r == 1:
                        ps = sc_tile((P, F), f32, name="scc")
                    else:
                        ps = psum.tile([P, F], f32, name="xpc", tag="xps",
                                       bufs=XPS_BUFS)
                    for jj in range(4):
                        nc.tensor.matmul(
                            ps, lhsT=x_all[:, jj, t * P:(t + 1) * P],
                            rhs=wo_sb[:, jj, n * F:(n + 1) * F],
                            start=(jj == 0), stop=(jj == 3),
                        )
                    ob = outp.tile([P, F], bf16, name="ob", tag="ob")
                    nc.vector.tensor_copy(out=ob, in_=ps)
                    q_eng = nc.sync if n == 0 else nc.scalar
                    q_eng.dma_start(out=out_view[:, t, n * F:(n + 1) * F], in_=ob)

    return nc


def _get_nc(flags):
    if len(flags) == 3:
        flags = (*flags, True)
    key = flags
    if key not in _CACHE:
        nc = _build(*flags)
        patched, _n = _split_excess_waits(nc.to_json_bytes())
        nc.to_json_bytes = lambda: patched
        _CACHE[key] = nc
    return _CACHE[key]


def _bf(a):
    return np.ascontiguousarray(np.asarray(a).astype(BF16))


def kernel(query, key, value, mask, Wq, bq, Wk, bk, Wv, bv, Wo, bo,
           q_gamma, q_beta, k_gamma, k_beta, _trace=False):
    from concourse.bass_utils import run_bass_kernel_spmd

    query = np.asarray(query, np.float32)
    key = np.asarray(key, np.float32)
    value = np.asarray(value, np.float32)
    mask = np.asarray(mask)
    Wq, Wk, Wv, Wo = (np.asarray(w, np.float32) for w in (Wq, Wk, Wv, Wo))
    bq, bk, bv, bo = (np.asarray(b, np.float32) for b in (bq, bk, bv, bo))
    q_gamma, q_beta, k_gamma, k_beta = (
        np.asarray(g, np.float32) for g in (q_gamma, q_beta, k_gamma, k_beta))

    B = query.shape[0]
    use_bq, use_bk, use_bv = (bool(np.any(b)) for b in (bq, bk, bv))
    ln_beta_zero = not (np.any(q_beta) or np.any(k_beta))
    nc = _get_nc((use_bq, use_bk, use_bv, ln_beta_zero))

    # host-side shard prep
    xq16 = [_bf(query[b].T) for b in range(B)]
    xk16 = [_bf(key[b].T) for b in range(B)]
    xv16 = [_bf(value[b].T) for b in range(B)]
    mask16 = [np.ascontiguousarray((~mask[b]).T.astype(BF16)) for b in range(B)]
    # per-partition LN consts (pair-local feature); q folds 1/sqrt(dk)=1/8
    def tile2(v):
        return np.ascontiguousarray(np.tile(v, 2).reshape(P, 1).astype(np.float32))
    consts = {
        "gq": tile2(q_gamma / 8.0),
        "gk": tile2(k_gamma),
    }
    if not ln_beta_zero:
        consts.update({
            "bq_ln": tile2(q_beta / 8.0),
            "bk_ln": tile2(k_beta),
        })

    in_maps = []
    for core in range(8):
        b, g = core // 2, core % 2
        sl = slice(g * F, (g + 1) * F)
        im = {
            "xq16": xq16[b], "xk16": xk16[b], "xv16": xv16[b],
            "wq16": _bf(Wq[sl].T),
            "wk16": _bf(Wk[sl].T),
            "wv16": _bf(Wv[sl].T),
            "wo16": _bf(Wo[:, sl].T),
            "mask16": mask16[b],
            **consts,
        }
        if use_bq:
            im["bq"] = np.ascontiguousarray(bq[sl])
        if use_bk:
            im["bk"] = np.ascontiguousarray(bk[sl])
        if use_bv:
            im["bv"] = np.ascontiguousarray(bv[sl])
        in_maps.append(im)

    res = run_bass_kernel_spmd(nc, in_maps, core_ids=list(range(8)), trace=_trace)
    out = np.zeros((B, T, D), np.float32)
    for b in range(B):
        out[b] = (res.results[2 * b]["out_p"].astype(np.float32)
                  + res.results[2 * b + 1]["out_p"].astype(np.float32) + bo)
    if _trace:
        kernel._last_results = res
    return out


# revision 41
# speedup vs baseline: 1.4379x; 1.0041x over previous
"""Trainium2 Bass kernel for nn_MultiHeadAttention (B=4, T=1024, D=1024, H=16, dk=64).

Sharding: 8 cores = 4 batches x 2 head-groups (8 heads / 512 features each).
Each core computes a partial output (its head-group's contribution through Wo);
host sums the two partials per batch and adds bo.

Per-core dataflow (one NeuronCore, Tile-scheduled), cost-model-guided; all
matmuls bf16 (fp8 DoubleRow was tried and rejected: e4m3's 2.65% RMS lands
~6% on the output, over the 2e-2 gate):
  A) q/k projections (bf16, 8 d-chunk accumulation) -> LN per head
     (bn_stats + bn_aggr, Rsqrt(var+eps) on ACT) -> normalize (Pool) ->
     qh bf16 -> 4 PE transposes per tile into a [P,512] PSUM quad ->
     one ACT scale-drain (gamma, and 1/sqrt(dk) on q) into qlnT/klnT
     [pair-features x 4 pairs x T].
  B) head-major attention: scoresT per (h, tk-block) = klnT-slice.T @ qlnT
     (2x 512-chunk matmuls, K=64) -> PSUM [128,1024]; exp on ACT (the
     bottleneck engine of this phase - nothing else is scheduled on ACT
     here); mask multiply on DVE; attnV transposed: out partitions = query
     positions, rhs = v_sb[tk,h,:] = [v|1] (65 cols, ones column yields
     softmax denominators in col 64) accumulated over tk chunks into
     bank-sized PSUM [128,4,128] tiles; DVE reciprocal + per-qb
     tensor_scalar -> xT bf16; after each odd head 8 PE transposes ->
     x_all [f, T]. v-projection is interleaved into head 0's tk loop to
     fill the ACT-bound window with PE work.
  C) out = x_all-slices.T @ Wo (bf16) -> (T,1024) f32 partial, DMA out
     (alternating sync/gpsimd queues to halve the tail).

PSUM (8 banks): pp 2 (proj/v/out + B-phase x-transpose quads), sc 2x2
(scores [P,1024]; also hosts phase-A transpose quads - disjoint phases),
xps 2 (attnV accumulators). start=True zeroes a whole 2KB bank, so
multi-region accumulation uses start only on the globally-first matmul and
stop on the last; skip_group_check silences the checker for interior ones.

walrus allows only ONE sync-wait per instruction (_split_excess_waits
patches the BIR).
"""

import numpy as np
import ml_dtypes

T = 1024
D = 1024
F = 512      # features per core (8 heads x 64)
NH = 8       # heads per core
DK = 64
P = 128
EPS = 1e-5
BF16 = ml_dtypes.bfloat16

_CACHE = {}

# tuning knobs
PP_BUFS = 2
SC_BUFS = 2
XPS_BUFS = 2
AT_BUFS = 13
XT_BUFS = 3
DRAIN_BUFS = 4
QH_BUFS = 4
STAT_BUFS = 13
OUT_BUFS = 6
DEFER_PAIR_TRANSPOSE = True  # emit pair transposes after next head's first scores


def _split_excess_waits(bj):
    """Walrus allows at most 1 sync-wait per instruction (2 for
    EventSemaphore). Tile's sem assigner can emit more; spill the excess
    onto NoOp carriers inserted just before, on the same engine."""
    import json
    d = json.loads(bj)
    ctr = 0
    for fn in d["functions"]:
        for bb in fn["blocks"]:
            new = []
            for inst in bb["instructions"]:
                si = inst.get("sync_info") or {}
                ow = si.get("on_wait") or []
                op = inst.get("opcode", "")
                cap = 2 if op == "EventSemaphore" else 1
                if len(ow) > cap:
                    for w in ow[:-cap]:
                        ctr += 1
                        new.append({
                            "debug": inst.get("debug", 0),
                            "engine": inst["engine"],
                            "ins": [], "outs": [],
                            "name": f"W-{ctr}",
                            "opcode": "NoOp",
                            "sync_info": {"on_update": [], "on_wait": [w]},
                            "text_hint": "waitsplit",
                        })
                    si["on_wait"] = ow[-cap:]
                new.append(inst)
            bb["instructions"] = new
    return json.dumps(d).encode(), ctr


def _build(use_bq, use_bk, use_bv, ln_beta_zero=True):
    import concourse.bass as bass
    import concourse.tile as tile
    from concourse import mybir
    from concourse.masks import make_identity

    f32 = mybir.dt.float32
    bf16 = mybir.dt.bfloat16
    ALU = mybir.AluOpType
    ACTF = mybir.ActivationFunctionType

    nc = bass.Bass()

    # ---- DRAM I/O ----
    xq_d = nc.dram_tensor("xq16", (D, T), bf16, kind="ExternalInput").ap()
    xk_d = nc.dram_tensor("xk16", (D, T), bf16, kind="ExternalInput").ap()
    xv_d = nc.dram_tensor("xv16", (D, T), bf16, kind="ExternalInput").ap()
    wq_d = nc.dram_tensor("wq16", (D, F), bf16, kind="ExternalInput").ap()
    wk_d = nc.dram_tensor("wk16", (D, F), bf16, kind="ExternalInput").ap()
    wv_d = nc.dram_tensor("wv16", (D, F), bf16, kind="ExternalInput").ap()
    wo_d = nc.dram_tensor("wo16", (F, D), bf16, kind="ExternalInput").ap()
    mask_d = nc.dram_tensor("mask16", (T, T), bf16, kind="ExternalInput").ap()
    # per-partition LN constants (128,1) = per pair-local feature
    gl_d = {}
    for nm in ("gq", "gk"):
        gl_d[nm] = nc.dram_tensor(nm, (P, 1), f32, kind="ExternalInput").ap()
    if not ln_beta_zero:
        for nm in ("bq_ln", "bk_ln"):
            gl_d[nm] = nc.dram_tensor(nm, (P, 1), f32, kind="ExternalInput").ap()
    biases = {}
    for name, used in (("bq", use_bq), ("bk", use_bk), ("bv", use_bv)):
        if used:
            biases[name] = nc.dram_tensor(name, (F,), f32, kind="ExternalInput").ap()
    out_p = nc.dram_tensor("out_p", (T, D), bf16, kind="ExternalOutput").ap()

    # DRAM views
    xviews = {
        "q": xq_d.rearrange("(dc p) t -> p dc t", p=P),
        "k": xk_d.rearrange("(dc p) t -> p dc t", p=P),
        "v": xv_d.rearrange("(dc p) t -> p dc t", p=P),
    }
    wviews = {
        "q": wq_d.rearrange("(dc p) f -> p dc f", p=P),
        "k": wk_d.rearrange("(dc p) f -> p dc f", p=P),
        "v": wv_d.rearrange("(dc p) f -> p dc f", p=P),
    }
    wo_view = wo_d.rearrange("(fc p) d -> p fc d", p=P)
    mask_view = mask_d.rearrange("(kc p) t -> p kc t", p=P)
    out_view = out_p.rearrange("(tc p) d -> p tc d", p=P)

    with tile.TileContext(nc) as tc:
        with (
            tc.tile_pool(name="const", bufs=1) as const,
            tc.tile_pool(name="drain", bufs=DRAIN_BUFS) as drain,
            tc.tile_pool(name="stat", bufs=STAT_BUFS) as stat,
            tc.tile_pool(name="qhatp", bufs=QH_BUFS) as qhatp,
            tc.tile_pool(name="attnp", bufs=AT_BUFS) as attnp,
            tc.tile_pool(name="xtp", bufs=XT_BUFS) as xtp,
            tc.tile_pool(name="recipp", bufs=4) as recipp,
            tc.tile_pool(name="outp", bufs=OUT_BUFS) as outp,
            tc.tile_pool(name="psum", bufs=1, space="PSUM") as psum,
        ):
            def pp_tile(shape=(P, F), dtype=f32, name="pp"):
                return psum.tile(list(shape), dtype, name=name, tag="pp",
                                 bufs=PP_BUFS)

            def sc_tile(shape=(P, T), dtype=f32, name="sc"):
                return psum.tile(list(shape), dtype, name=name, tag="sc",
                                 bufs=SC_BUFS)

            def xps_tile(g):
                return psum.tile([P, 4, P], f32, name=f"xps{g}", tag="xps",
                                 bufs=XPS_BUFS)

            # ---- resident tiles ----
            x_sb = {
                pn: const.tile([P, 8, T], bf16, name=f"x{pn}_sb", tag=f"x{pn}_sb")
                for pn in ("q", "k", "v")
            }
            w_sb = {
                pn: const.tile([P, 8, F], bf16, name=f"w{pn}_sb", tag=f"w{pn}_sb")
                for pn in ("q", "k", "v")
            }
            wo_sb = const.tile([P, 4, D], bf16, name="wo", tag="wo")
            mask_sb = const.tile([P, 8, T], bf16, name="mask", tag="mask")
            qlnT = const.tile([P, 4, T], bf16, name="qlnT", tag="qlnT")
            klnT = const.tile([P, 4, T], bf16, name="klnT", tag="klnT")
            v_sb = const.tile([P, 8, NH, 65], bf16, name="v_sb", tag="v_sb")
            x_all = const.tile([P, 4, T], bf16, name="x_all", tag="x_all")
            eps_t = const.tile([P, 1], f32, name="eps", tag="eps")
            gb_t = {}
            for nm, dr_ in gl_d.items():
                gb_t[nm] = const.tile([P, 1], f32, name=f"ln_{nm}", tag=f"ln_{nm}")
                nc.gpsimd.dma_start(gb_t[nm], dr_)
            nc.vector.memset(eps_t, EPS)
            ident16 = const.tile([P, P], bf16, name="ident16", tag="ident16")
            make_identity(nc, ident16)
            # ones column for softmax denominators
            nc.vector.memset(v_sb[:, :, :, 64:65], 1.0)


            bias_bc = {}
            for name in biases:
                bias_bc[name] = const.tile([P, F], f32, name=f"bc_{name}", tag=f"bc_{name}")
                src = bass.AP(
                    tensor=biases[name].tensor,
                    offset=biases[name].offset,
                    ap=[[0, P], [1, F]],
                )
                nc.gpsimd.dma_start(out=bias_bc[name], in_=src)

            # ---- input DMAs ----
            # sync queue, ordered so the k-projection can start ASAP
            def load_x(pn, quarters):
                for qtr in quarters:
                    sl = slice(qtr * 256, (qtr + 1) * 256)
                    nc.sync.dma_start(x_sb[pn][:, :, sl], xviews[pn][:, :, sl])

            nc.sync.dma_start(w_sb["k"][:, 0:1, :], wviews["k"][:, 0:1, :])
            load_x("k", range(1))
            for d in range(1, 8):
                nc.sync.dma_start(w_sb["k"][:, d:d + 1, :], wviews["k"][:, d:d + 1, :])
            load_x("k", range(1, 4))
            nc.sync.dma_start(w_sb["q"], wviews["q"])
            load_x("q", range(2))
            nc.sync.dma_start(w_sb["v"], wviews["v"])
            load_x("q", range(2, 4))
            load_x("v", range(4))
            nc.sync.dma_start(wo_sb, wo_view)
            # mask via the SWDGE queue, concurrent with the sync queue
            for half in range(2):
                nc.gpsimd.dma_start(mask_sb[:, 4 * half:4 * half + 4, :],
                                    mask_view[:, 4 * half:4 * half + 4, :])

            # ---- Phase A: q/k projections + LN + transpose ----
            a_pending = []

            def flush_a():
                while a_pending:
                    a_pending.pop(0)()

            def proj_ln(pn, dstT, t):
                bias_name = "b" + pn
                ps = pp_tile()
                for d in range(8):
                    nc.tensor.matmul(
                        ps, lhsT=x_sb[pn][:, d, t * P:(t + 1) * P],
                        rhs=w_sb[pn][:, d, :],
                        start=(d == 0), stop=(d == 7),
                    )
                # deferred transposes/gamma-drains of older tiles go here:
                # after this tile's matmuls (PE) and before its drain (ACT),
                # so neither engine's in-order queue blocks them.
                while len(a_pending) > 3:
                    a_pending.pop(0)()
                sb = drain.tile([P, NH, DK], f32, name="qsb", tag="qsb")
                if bias_name in bias_bc:
                    nc.vector.tensor_add(
                        sb.rearrange("p h d -> p (h d)"), ps, bias_bc[bias_name])
                else:
                    nc.scalar.activation(
                        out=sb.rearrange("p h d -> p (h d)"), in_=ps,
                        func=ACTF.Copy)
                st = stat.tile([P, NH, 6], f32, name="st", tag="st")
                for h in range(NH):
                    nc.vector.bn_stats(out=st[:, h, :], in_=sb[:, h, :])
                ag = stat.tile([P, NH, 2], f32, name="ag", tag="ag")
                for h in range(NH):
                    nc.vector.bn_aggr(out=ag[:, h, :], in_=st[:, h, :])
                sd = stat.tile([P, NH], f32, name="sd", tag="sd")
                nc.scalar.activation(
                    out=sd, in_=ag[:, :, 1], func=ACTF.Sqrt, bias=eps_t)
                rs = stat.tile([P, NH], f32, name="rs", tag="rs")
                nc.vector.reciprocal(out=rs, in_=sd)
                qh = qhatp.tile([P, F], bf16, name="qh", tag="qh")
                for h in range(NH):
                    nc.gpsimd.tensor_scalar(
                        out=qh[:, h * DK:(h + 1) * DK],
                        in0=sb[:, h, :],
                        scalar1=ag[:, h, 0:1],
                        scalar2=rs[:, h:h + 1],
                        op0=ALU.subtract,
                        op1=ALU.mult,
                    )
                # 4 pair-transposes into one [P,512] PSUM quad (on the sc
                # tag: scores don't run during phase A), then ONE gamma
                # scale-drain. Deferred one tile so the PE doesn't wait on
                # the LN chain.
                def emit(qh=qh, pn=pn, dstT=dstT, t=t):
                    pst4 = sc_tile((P, 4, P), bf16, name="pst4")
                    for j in range(4):
                        nc.tensor.transpose(
                            pst4[:, j, :], qh[:, j * P:(j + 1) * P], ident16)
                    g_nm, b_nm = ("gq", "bq_ln") if pn == "q" else ("gk", "bk_ln")
                    dst = dstT[:, :, t * P:(t + 1) * P]
                    if ln_beta_zero:
                        nc.scalar.activation(
                            out=dst, in_=pst4, func=ACTF.Copy, scale=gb_t[g_nm])
                    else:
                        nc.scalar.tensor_scalar(
                            out=dst, in0=pst4,
                            scalar1=gb_t[g_nm], scalar2=gb_t[b_nm],
                            op0=ALU.mult, op1=ALU.add)
                a_pending.append(emit)

            order = [("k", klnT, t) for t in range(8)] + \
                [("q", qlnT, t) for t in range(8)]
            for pn, dstT, t in order:
                proj_ln(pn, dstT, t)
            while a_pending:
                a_pending.pop(0)()

            # ---- Phase B: attention, one flat software pipeline ----
            # Per unit (h, tk): emit scores/exp/mask, then the PREVIOUS
            # unit's attnV matmuls, so the PE never waits on exp+mask.
            # Head drains (reciprocal + scale) and pair transposes are
            # emitted when that head's last attnV retires.
            pending = []  # deferred pair-transpose emitters

            def flush_pending():
                while pending:
                    pending.pop(0)()

            xps_h = {}
            xTb_h = {}

            def attn_v(h, tk, at):
                xps = xps_h[h]
                for qg in range(2):
                    for qb in range(4):
                        j = qg * 4 + qb
                        first = (tk == 0 and qb == 0)
                        last = (tk == 7 and qb == 3)
                        nc.tensor.matmul(
                            xps[qg][:, qb, 0:65],
                            lhsT=at[:, j * P:(j + 1) * P],
                            rhs=v_sb[:, tk, h, :],
                            start=first, stop=last,
                            skip_group_check=not (first or last),
                        )

            def head_drain(h):
                xps = xps_h.pop(h)
                if h % 2 == 0:
                    xTb_h[h // 2] = xtp.tile([P, 8, P], bf16, name="xTb", tag="xTb")
                xTb = xTb_h[h // 2]
                csl = slice(0, DK) if h % 2 == 0 else slice(DK, P)
                for qg in range(2):
                    rc = recipp.tile([P, 4], f32, name="rc", tag="rc")
                    nc.vector.reciprocal(out=rc, in_=xps[qg][:, :, 64:65])
                    for qb in range(4):
                        nc.vector.tensor_scalar(
                            out=xTb[:, qg * 4 + qb, csl],
                            in0=xps[qg][:, qb, 0:64],
                            scalar1=rc[:, qb:qb + 1], scalar2=None,
                            op0=ALU.mult)
                if h % 2 == 1:
                    jj = h // 2

                    def emit_transposes(xTb=xTb, jj=jj):
                        for qg in range(2):
                            pst4 = pp_tile((P, 4, P), bf16, name="pstx")
                            for qb in range(4):
                                nc.tensor.transpose(
                                    pst4[:, qb, :], xTb[:, qg * 4 + qb, :], ident16)
                            nc.vector.tensor_copy(
                                out=x_all[:, jj, qg * F:(qg + 1) * F],
                                in_=pst4.rearrange("p a b -> p (a b)"))
                    if DEFER_PAIR_TRANSPOSE and h < NH - 1:
                        pending.append(emit_transposes)
                    else:
                        emit_transposes()

            at_q = []
            v_tasks = []  # (tk, d_lo, d_hi, drain?) chunks, 4 matmuls each
            for tk in range(8):
                v_tasks.append((tk, 0, 4, False))
                v_tasks.append((tk, 4, 8, True))
            v_ps = {}

            def v_chunk():
                tk, dlo, dhi, do_drain = v_tasks.pop(0)
                if dlo == 0:
                    v_ps[tk] = pp_tile()
                ps = v_ps[tk]
                for d in range(dlo, dhi):
                    nc.tensor.matmul(
                        ps, lhsT=x_sb["v"][:, d, tk * P:(tk + 1) * P],
                        rhs=w_sb["v"][:, d, :],
                        start=(d == 0), stop=(d == 7),
                    )
                if do_drain:
                    ps = v_ps.pop(tk)
                    if "bv" in bias_bc:
                        vb = drain.tile([P, NH, DK], f32, name="vsb", tag="qsb")
                        nc.vector.tensor_add(
                            vb.rearrange("p h d -> p (h d)"), ps, bias_bc["bv"])
                        nc.gpsimd.tensor_copy(out=v_sb[:, tk, :, 0:64], in_=vb)
                    else:
                        nc.vector.tensor_copy(
                            out=v_sb[:, tk, :, 0:64],
                            in_=ps.rearrange("p (h c) -> p h c", c=DK))

            def pop_attnv():
                hp, tkp, atp_ = at_q.pop(0)
                attn_v(hp, tkp, atp_)
                if tkp == 7:
                    head_drain(hp)
                if hp % 2 == 1 and tkp == 1:
                    flush_pending()

            u = 0
            for h in range(NH):
                rows = slice((h % 2) * DK, (h % 2) * DK + DK)
                pair = h // 2
                xps_h[h] = [xps_tile(g) for g in range(2)]
                for tk in range(8):
                    # v-projection: one 4-matmul chunk per unit over the
                    # first 16 units; h=0's attnV lags 9 units so v(tk) is
                    # always emitted before its consumer is popped
                    if v_tasks:
                        v_chunk()
                    sp = sc_tile()
                    for n in range(2):
                        nc.tensor.matmul(
                            sp[:, n * F:(n + 1) * F],
                            lhsT=klnT[rows, pair, tk * P:(tk + 1) * P],
                            rhs=qlnT[rows, pair, n * F:(n + 1) * F],
                            start=True, stop=True,
                        )
                    at = attnp.tile([P, T], bf16, name="at", tag="at")
                    nc.scalar.activation(out=at, in_=sp, func=ACTF.Exp)
                    nc.vector.tensor_mul(at, at, mask_sb[:, tk, :])
                    at_q.append((h, tk, at))
                    target = 9 if u < 16 else max(1, 9 - (u - 16) // 2)
                    while len(at_q) > target:
                        pop_attnv()
                    u += 1
            while at_q:
                pop_attnv()
            flush_pending()

            # ---- Phase C: output projection ----
            for t in range(8):
                for n in range(2):
                    r = (2 * t + n) % 3
                    if r == 0:
                        ps = pp_tile()
                    elif r == 1:
                        ps = sc_tile((P, F), f32, name="scc")
                    else:
                        ps = psum.tile([P, F], f32, name="xpc", tag="xps",
                                       bufs=XPS_BUFS)
                    for jj in range(4):
                        nc.tensor.matmul(
                            ps, lhsT=x_all[:, jj, t * P:(t + 1) * P],
                            rhs=wo_sb[:, jj, n * F:(n + 1) * F],
                            start=(jj == 0), stop=(jj == 3),
                        )
                    ob = outp.tile([P, F], bf16, name="ob", tag="ob")
                    nc.vector.tensor_copy(out=ob, in_=ps)
                    q_eng = nc.sync if n == 0 else nc.scalar
                    q_eng.dma_start(out=out_view[:, t, n * F:(n + 1) * F], in_=ob)

    return nc


def _get_nc(flags):
    if len(flags) == 3:
        flags = (*flags, True)
    key = flags
    if key not in _CACHE:
        nc = _build(*flags)
        patched, _n = _split_excess_waits(nc.to_json_bytes())
        nc.to_json_bytes = lambda: patched
        _CACHE[key] = nc
    return _CACHE[key]


def _bf(a):
    return np.ascontiguousarray(np.asarray(a).astype(BF16))


def kernel(query, key, value, mask, Wq, bq, Wk, bk, Wv, bv, Wo, bo,
           q_gamma, q_beta, k_gamma, k_beta, _trace=False):
    from concourse.bass_utils import run_bass_kernel_spmd

    query = np.asarray(query, np.float32)
    key = np.asarray(key, np.float32)
    value = np.asarray(value, np.float32)
    mask = np.asarray(mask)
    Wq, Wk, Wv, Wo = (np.asarray(w, np.float32) for w in (Wq, Wk, Wv, Wo))
    bq, bk, bv, bo = (np.asarray(b, np.float32) for b in (bq, bk, bv, bo))
    q_gamma, q_beta, k_gamma, k_beta = (
        np.asarray(g, np.float32) for g in (q_gamma, q_beta, k_gamma, k_beta))

    B = query.shape[0]
    use_bq, use_bk, use_bv = (bool(np.any(b)) for b in (bq, bk, bv))
    ln_beta_zero = not (np.any(q_beta) or np.any(k_beta))
    nc = _get_nc((use_bq, use_bk, use_bv, ln_beta_zero))

    # host-side shard prep
    xq16 = [_bf(query[b].T) for b in range(B)]
    xk16 = [_bf(key[b].T) for b in range(B)]
    xv16 = [_bf(value[b].T) for b in range(B)]
    mask16 = [np.ascontiguousarray((~mask[b]).T.astype(BF16)) for b in range(B)]
    # per-partition LN consts (pair-local feature); q folds 1/sqrt(dk)=1/8
    def tile2(v):
        return np.ascontiguousarray(np.tile(v, 2).reshape(P, 1).astype(np.float32))
    consts = {
        "gq": tile2(q_gamma / 8.0),
        "gk": tile2(k_gamma),
    }
    if not ln_beta_zero:
        consts.update({
            "bq_ln": tile2(q_beta / 8.0),
            "bk_ln": tile2(k_beta),
        })

    in_maps = []
    for core in range(8):
        b, g = core // 2, core % 2
        sl = slice(g * F, (g + 1) * F)
        im = {
            "xq16": xq16[b], "xk16": xk16[b], "xv16": xv16[b],
            "wq16": _bf(Wq[sl].T),
            "wk16": _bf(Wk[sl].T),
            "wv16": _bf(Wv[sl].T),
            "wo16": _bf(Wo[:, sl].T),
            "mask16": mask16[b],
            **consts,
        }
        if use_bq:
            im["bq"] = np.ascontiguousarray(bq[sl])
        if use_bk:
            im["bk"] = np.ascontiguousarray(bk[sl])
        if use_bv:
            im["bv"] = np.ascontiguousarray(bv[sl])
        in_maps.append(im)

    res = run_bass_kernel_spmd(nc, in_maps, core_ids=list(range(8)), trace=_trace)
    out = np.zeros((B, T, D), np.float32)
    for b in range(B):
        out[b] = (res.results[2 * b]["out_p"].astype(np.float32)
                  + res.results[2 * b + 1]["out_p"].astype(np.float32) + bo)
    if _trace:
        kernel._last_results = res
    return out
